# revision 5
# baseline (speedup 1.0000x reference)
"""BertAttention Trainium2 kernel — 8-core SPMD, v2.

Sharding: each core owns 2 heads (128 of the 1024 feature dims) and a
512-token output slice (4 x 128-token quarters, one per 1024-token span).

Key design points (vs v1):
  - Global slot schedule: one 128-key score chunk per slot; ctx matmuls
    lag scores by 2 slots (h0) / 3 slots (h1); exp (ACT engine) is one
    slot behind scores with double-buffered PSUM so the PE never stalls
    on the activation engine and keeps its fast p-state.
  - QKV projections for batch 1 and the output projection run as filler
    matmul units inside attention slots that would otherwise idle.
  - Four quarter AllToAlls (256KB each) replace the AllGather (7MB/core):
    each core receives exactly the full-width ctx^T for its own tokens,
    so phase 3 is rank-static and overlaps with attention.

Host passes activations/weights pre-transposed and pre-cast to bf16
(feature-major), so the device spends no time on casts/transposes.
"""

import os
import sys

for _p in ("/opt/trn_rl_repo", "/root/.axon_site/_ro/trn_rl_repo"):
    if os.path.isdir(_p) and _p not in sys.path:
        sys.path.append(_p)

import ml_dtypes
import numpy as np

# Shim antenv.axon_hooks (absent in some images): bass_utils imports it
# unconditionally when tracing is requested via env.
try:
    import antenv.axon_hooks  # noqa: F401
except Exception:
    import types as _types
    try:
        import antenv as _antenv
        _m = _types.ModuleType("antenv.axon_hooks")
        _m._hook = None
        _m.set_axon_ntff_profile_hook = lambda h, _m=_m: setattr(_m, "_hook", h)
        _m.get_axon_ntff_profile_hook = lambda _m=_m: _m._hook
        sys.modules["antenv.axon_hooks"] = _m
        _antenv.axon_hooks = _m
    except Exception:
        pass

import concourse.bass as bass  # noqa: F401
import concourse.tile as tile
from concourse import bacc, mybir
from concourse.bass_utils import run_bass_kernel_spmd

F32 = mybir.dt.float32
BF16 = mybir.dt.bfloat16
BF16_NP = ml_dtypes.bfloat16

NCORES = 8
H = 16   # heads total
DH = 64  # head dim
LN_EPS = 1e-12


def build_bert_kernel(S=2048, B=2, D=1024):
    P = 128
    NTOK = S * B              # 4096 batch-major tokens
    TPC = NTOK // NCORES      # 512 output tokens per core (4 quarters)
    CCH = D // P              # 8 contraction chunks
    HPC = H // NCORES         # 2 heads per core
    DL = HPC * DH             # 128 local feature dims
    NJ = S // P               # 16 key chunks per batch
    NI = S // 512             # 4 query blocks per batch
    NSLOT = B * NI * NJ       # 128 score chunk-slots
    NQTR = NTOK // 1024       # 4 quarters
    NVT = NTOK // P           # 32 v token tiles

    nc = bacc.Bacc("TRN2", target_bir_lowering=False, debug=False,
                   num_devices=NCORES)

    def din(name, shape, dt=F32):
        return nc.dram_tensor(name, list(shape), dt, kind="ExternalInput").ap()

    xqT = din("xqT", (D, NTOK), BF16)
    xkT = din("xkT", (D, NTOK), BF16)
    xvT = din("xvT", (D, NTOK), BF16)
    wqT = din("wqT", (D, DL), BF16)
    wkT = din("wkT", (D, DL), BF16)
    wvT = din("wvT", (D, DL), BF16)
    woT = din("woT", (D, D), BF16)
    bq = din("bq", (DL, 1))
    bk = din("bk", (DL, 1))
    bv = din("bv", (1, DL))
    bo = din("bo", (1, D))
    lnw = din("lnw", (1, D))
    lnb = din("lnb", (1, D))
    resid = din("resid", (TPC, D))
    out = nc.dram_tensor("out", [TPC, D], F32, kind="ExternalOutput").ap()

    # per-quarter exchange buffers: piece p of a2a[q] = ctx^T of this
    # core's heads for tokens [q*1024 + p*128, +128) -> destined core p.
    a2a = [nc.dram_tensor(f"a2a{q}", [NCORES, P, P], BF16).ap()
           for q in range(NQTR)]
    ag = [nc.dram_tensor(f"ag{q}", [NCORES, P, P], BF16).ap()
          for q in range(NQTR)]
    GRP = [list(range(NCORES))]

    with tile.TileContext(nc) as tc:
        with (
            tc.tile_pool(name="persist", bufs=1) as persist,
            tc.tile_pool(name="small", bufs=1) as small,
            tc.tile_pool(name="xT", bufs=1) as xt_pool,
            tc.tile_pool(name="work", bufs=1) as work,
            tc.tile_pool(name="ps_sc", bufs=1, space="PSUM") as ps_sc,
            tc.tile_pool(name="ps_cps", bufs=1, space="PSUM") as ps_cps,
            tc.tile_pool(name="ps_fill", bufs=1, space="PSUM") as ps_fill,
        ):
            # ---- weights into SBUF ----
            wqT_sb = persist.tile([P, CCH, DL], BF16)
            wkT_sb = persist.tile([P, CCH, DL], BF16)
            wvT_sb = persist.tile([P, CCH, DL], BF16)
            for wi, (w_d, w_sb) in enumerate(
                    ((wqT, wqT_sb), (wkT, wkT_sb), (wvT, wvT_sb))):
                for c in range(CCH):
                    (nc.sync if (wi + c) % 2 else nc.gpsimd).dma_start(
                        out=w_sb[:, c, :], in_=w_d[c * P:(c + 1) * P, :])
            woT_sb = persist.tile([P, CCH, D], BF16)
            for c in range(CCH):
                nc.sync.dma_start(out=woT_sb[:, c, :],
                                  in_=woT[c * P:(c + 1) * P, :])

            # ---- constant / bias tiles ----
            bq_sb = small.tile([P, 1], F32)
            bk_sb = small.tile([P, 1], F32)
            nc.sync.dma_start(out=bq_sb, in_=bq)
            nc.sync.dma_start(out=bk_sb, in_=bk)
            bv_bc = small.tile([P, DL], F32)
            nc.gpsimd.dma_start(out=bv_bc, in_=bv.to_broadcast((P, DL)))
            bo_bc = small.tile([P, D], F32)
            nc.gpsimd.dma_start(out=bo_bc, in_=bo.to_broadcast((P, D)))
            lnw_bc = small.tile([P, D], F32)
            nc.gpsimd.dma_start(out=lnw_bc, in_=lnw.to_broadcast((P, D)))
            lnb_bc = small.tile([P, D], F32)
            nc.gpsimd.dma_start(out=lnb_bc, in_=lnb.to_broadcast((P, D)))
            eps_sb = small.tile([P, 1], F32)
            nc.vector.memset(eps_sb, LN_EPS)

            # ---- persistent activation buffers ----
            qT_sb = persist.tile([P, NTOK], BF16)   # [dloc, tok]
            kT_sb = persist.tile([P, NTOK], BF16)
            v_sb = persist.tile([P, NVT, HPC * (DH + 1)], BF16)
            nc.vector.memset(v_sb[:, :, DH:DH + 1], 1.0)
            nc.vector.memset(v_sb[:, :, 2 * DH + 1:2 * DH + 2], 1.0)
            e_ring = persist.tile([P, 4, 2 * 512], BF16)  # exp ring, %4
            ctxF = [persist.tile([P, CCH, P], BF16, name=f"ctxF{q}")
                    for q in range(NQTR)]
            y_sb = [persist.tile([P, D], F32, name=f"y{q}")
                    for q in range(NQTR)]

            # ---- x tiles stream through a ring ----
            xt = {}
            ei = 0
            for b in range(B):
                for ti, x_d in enumerate((xqT, xkT, xvT)):
                    for c in range(CCH):
                        t = xt_pool.tile([P, S], BF16, name=f"xT{ti}_{c}_{b}",
                                         tag="xT", bufs=16)
                        xt[(ti, c, b)] = t
                        (nc.sync if ei % 2 else nc.gpsimd).dma_start(
                            out=t, in_=x_d[c * P:(c + 1) * P,
                                           b * S:(b + 1) * S])
                        ei += 1

            # ================= unit builders =================
            def qk_unit(ti, b, n):
                # projection of 512 tokens onto this core's 128 q/k dims
                w_sb, b_sb, o_sb = ((wqT_sb, bq_sb, qT_sb),
                                    (wkT_sb, bk_sb, kT_sb))[ti]
                ps = ps_fill.tile([P, 512], F32, tag="fill", bufs=2)
                for c in range(CCH):
                    nc.tensor.matmul(ps, w_sb[:, c, :],
                                     xt[(ti, c, b)][:, n * 512:(n + 1) * 512],
                                     start=(c == 0), stop=(c == CCH - 1))
                tok0 = b * S + n * 512
                nc.vector.tensor_scalar_add(o_sb[:, tok0:tok0 + 512], ps, b_sb)

            def v_unit(b, nt):
                # v projection for 4 token tiles (512 tokens)
                ps = ps_fill.tile([P, 512], F32, tag="fill", bufs=2)
                for k in range(4):
                    itl = nt * 4 + k
                    for c in range(CCH):
                        nc.tensor.matmul(
                            ps[:, k * P:(k + 1) * P],
                            xt[(2, c, b)][:, itl * P:(itl + 1) * P],
                            wvT_sb[:, c, :],
                            start=(c == 0), stop=(c == CCH - 1))
                for k in range(4):
                    it = b * (S // P) + nt * 4 + k
                    dst = v_sb[:, it, :].rearrange(
                        "p (h x) -> p h x", h=HPC)[:, :, 0:DH]
                    src = ps[:, k * P:(k + 1) * P].rearrange(
                        "p (h x) -> p h x", h=HPC)
                    bvr = bv_bc.rearrange("p (h x) -> p h x", h=HPC)
                    nc.vector.tensor_add(dst, src, bvr)

            resid_sb = {}

            def op_unit(i2, n):
                # output projection: 128 tokens x 512 output dims
                cF = ctxF[i2]
                ps = ps_fill.tile([P, 512], F32, tag="fill", bufs=2)
                for c in range(CCH):
                    nc.tensor.matmul(ps, cF[:, c, :],
                                     woT_sb[:, c, n * 512:(n + 1) * 512],
                                     start=(c == 0), stop=(c == CCH - 1))
                if n == 0:
                    rt = work.tile([P, D], F32, tag="resid", bufs=2,
                                   name=f"resid{i2}")
                    resid_sb[i2] = rt
                    nc.sync.dma_start(out=rt,
                                      in_=resid[i2 * P:(i2 + 1) * P, :])
                sl = slice(n * 512, (n + 1) * 512)
                y = y_sb[i2]
                nc.vector.tensor_add(y[:, sl], ps, bo_bc[:, sl])
                nc.vector.tensor_add(y[:, sl], y[:, sl], resid_sb[i2][:, sl])

            def ln_unit(i2):
                y = y_sb[i2]
                y3 = y.rearrange("p (g d) -> p g d", g=2)
                stats = work.tile([P, 2, 6], F32, tag="stats", bufs=2)
                for g in range(2):
                    nc.vector.bn_stats(out=stats[:, g, :], in_=y3[:, g, :])
                mv = work.tile([P, 2], F32, tag="mv", bufs=2)
                nc.vector.bn_aggr(out=mv, in_=stats)
                std = work.tile([P, 1], F32, tag="std", bufs=2)
                nc.scalar.activation(std, mv[:, 1:2],
                                     mybir.ActivationFunctionType.Sqrt,
                                     bias=eps_sb)
                rstd = work.tile([P, 1], F32, tag="rstd", bufs=2)
                nc.vector.reciprocal(rstd, std)
                t32 = work.tile([P, D], F32, tag="t32", bufs=2)
                nc.vector.tensor_scalar(
                    out=t32, in0=y, scalar1=mv[:, 0:1], scalar2=rstd,
                    op0=mybir.AluOpType.subtract, op1=mybir.AluOpType.mult)
                of = work.tile([P, D], F32, tag="of", bufs=2)
                nc.vector.tensor_mul(of, t32, lnw_bc)
                nc.vector.tensor_add(of, of, lnb_bc)
                nc.sync.dma_start(out=out[i2 * P:(i2 + 1) * P, :], in_=of)

            # ---- filler placement: slot -> list of unit closures ----
            fillers = {}

            def add_filler(s, fn):
                fillers.setdefault(s, []).append(fn)

            qkv_b1 = ([lambda n=n: qk_unit(1, 1, n) for n in range(4)]
                      + [lambda n=n: qk_unit(0, 1, n) for n in range(4)]
                      + [lambda nt=nt: v_unit(1, nt) for nt in range(4)])
            for u, fn in enumerate(qkv_b1):
                add_filler(3 + 4 * u, fn)          # slots 3..47
            add_filler(52, lambda: op_unit(0, 0))
            add_filler(56, lambda: op_unit(0, 1))
            add_filler(72, lambda: op_unit(1, 0))
            add_filler(76, lambda: op_unit(1, 1))
            add_filler(102, lambda: op_unit(2, 0))
            add_filler(106, lambda: op_unit(2, 1))

            # ================= main slot loop =================
            cps = [None, None]        # live ctx PSUM tile per head
            ctxo = {}                 # (b, i) -> staging tile

            def scores_step(s):
                b, r = divmod(s, NI * NJ)
                i, j = divmod(r, NJ)
                sc = ps_sc.tile([P, 2 * 512], F32, tag="sc", bufs=2)
                jc0 = b * S + j * P
                ic0 = b * S + i * 512
                for h in range(HPC):
                    nc.tensor.matmul(
                        sc[:, h * 512:(h + 1) * 512],
                        kT_sb[h * DH:(h + 1) * DH, jc0:jc0 + P],
                        qT_sb[h * DH:(h + 1) * DH, ic0:ic0 + 512])
                nc.scalar.activation(e_ring[:, s % 4, :], sc,
                                     mybir.ActivationFunctionType.Exp)

            def ctx_step(c, h):
                b, r = divmod(c, NI * NJ)
                i, j = divmod(r, NJ)
                vt = b * NJ + j
                if j == 0:
                    cps[h] = ps_cps.tile([DH + 1, 512], F32, name=f"cps{h}",
                                         tag=f"cps{h}", bufs=1)
                nc.tensor.matmul(
                    cps[h],
                    v_sb[:, vt, h * (DH + 1):(h + 1) * (DH + 1)],
                    e_ring[:, c % 4, h * 512:(h + 1) * 512],
                    start=(j == 0), stop=(j == NJ - 1))
                if j != NJ - 1:
                    return
                # ---- softmax normalize + stage for exchange ----
                if h == 0:
                    ctxo[(b, i)] = work.tile([P, 512], BF16, tag="ctxo",
                                             bufs=2, name=f"ctxo{b}_{i}")
                co = ctxo[(b, i)]
                rcp = work.tile([1, 512], F32, tag="rcp", bufs=2)
                nc.vector.reciprocal(rcp, cps[h][DH:DH + 1, :])
                rbc = work.tile([DH, 512], F32, tag="rbc", bufs=2)
                nc.gpsimd.partition_broadcast(rbc, rcp)
                nc.vector.tensor_mul(co[h * DH:(h + 1) * DH, :],
                                     cps[h][0:DH, :], rbc)
                if h == HPC - 1:
                    tok0 = b * S + i * 512
                    q = tok0 // 1024
                    p0 = (tok0 % 1024) // P
                    for t4 in range(4):
                        nc.sync.dma_start(
                            out=a2a[q][p0 + t4, :, :],
                            in_=co[:, t4 * P:(t4 + 1) * P])
                    if i % 2 == 1:  # quarter complete -> exchange
                        nc.gpsimd.collective_compute(
                            "AllToAll", mybir.AluOpType.bypass,
                            replica_groups=GRP,
                            ins=[a2a[q].opt()], outs=[ag[q].opt()])
                        for cc in range(CCH):
                            nc.sync.dma_start(out=ctxF[q][:, cc, :],
                                              in_=ag[q][cc, :, :])

            # prologue: QKV for batch 0 (solid PE block, ramps p-state)
            for n in range(4):
                qk_unit(1, 0, n)   # k first: scores need k chunks early
            for n in range(4):
                qk_unit(0, 0, n)
            for nt in range(4):
                v_unit(0, nt)

            for s in range(NSLOT + 3):
                if s < NSLOT:
                    scores_step(s)
                if 0 <= s - 2 < NSLOT:
                    ctx_step(s - 2, 0)
                if 0 <= s - 3 < NSLOT:
                    ctx_step(s - 3, 1)
                for fn in fillers.get(s, ()):
                    fn()

            # epilogue: last quarter's projection + all LayerNorms
            op_unit(3, 0)
            op_unit(3, 1)
            for i2 in range(NQTR):
                ln_unit(i2)

    nc.compile()
    return nc


_NC_CACHE = {}


def _get_nc(S=2048, B=2, D=1024):
    key = (S, B, D)
    if key not in _NC_CACHE:
        _NC_CACHE[key] = build_bert_kernel(S, B, D)
    return _NC_CACHE[key]


def make_in_maps(query_tensor, key_tensor, value_tensor, Wq, bq, Wk, bk,
                 Wv, bv, Wo, bo, ln_w, ln_b):
    S, B, D = query_tensor.shape
    NTOK = S * B
    DL = (H // NCORES) * DH

    def bm(x):  # (S, B, D) -> batch-major (B*S, D) float32
        return np.ascontiguousarray(
            np.asarray(x, np.float32).transpose(1, 0, 2).reshape(NTOK, D))

    def bmT(x):  # feature-major bf16 (D, B*S)
        return np.ascontiguousarray(bm(x).T.astype(BF16_NP))

    xq = bm(query_tensor)
    xqT, xkT, xvT = bmT(query_tensor), bmT(key_tensor), bmT(value_tensor)
    woT = np.ascontiguousarray(
        np.asarray(Wo, np.float32).T.astype(BF16_NP))
    f32 = lambda a: np.ascontiguousarray(np.asarray(a, np.float32))
    bf16T = lambda a: np.ascontiguousarray(
        np.asarray(a, np.float32).T.astype(BF16_NP))
    in_maps = []
    for c in range(NCORES):
        sl = slice(c * DL, (c + 1) * DL)
        rs = np.concatenate([xq[q * 1024 + c * 128:q * 1024 + (c + 1) * 128]
                             for q in range(NTOK // 1024)], axis=0)
        in_maps.append({
            "xqT": xqT, "xkT": xkT, "xvT": xvT,
            "wqT": bf16T(Wq[sl]), "wkT": bf16T(Wk[sl]),
            "wvT": bf16T(Wv[sl]), "woT": woT,
            "bq": f32(bq[sl]).reshape(DL, 1),
            "bk": f32(bk[sl]).reshape(DL, 1),
            "bv": f32(bv[sl]).reshape(1, DL),
            "bo": f32(bo).reshape(1, D),
            "lnw": f32(ln_w).reshape(1, D),
            "lnb": f32(ln_b).reshape(1, D),
            "resid": np.ascontiguousarray(rs),
        })
    return in_maps


def assemble_output(results, S, B, D):
    NTOK = S * B
    full = np.empty((NTOK, D), np.float32)
    for c, r in enumerate(results):
        o = r["out"]  # (512, D): 4 x 128-token quarter slices
        for q in range(NTOK // 1024):
            full[q * 1024 + c * 128:q * 1024 + (c + 1) * 128] = \
                o[q * 128:(q + 1) * 128]
    return np.ascontiguousarray(
        full.reshape(B, S, D).transpose(1, 0, 2))


def kernel(**inputs):
    S, B, D = inputs["query_tensor"].shape
    nc = _get_nc(S, B, D)
    in_maps = make_in_maps(**inputs)
    res = run_bass_kernel_spmd(nc, in_maps, list(range(NCORES)))
    return assemble_output(res.results, S, B, D)


# revision 23
# speedup vs baseline: 1.0272x; 1.0272x over previous
"""BertAttention Trainium2 kernel — 8-core SPMD, v2.

Sharding: each core owns 2 heads (128 of the 1024 feature dims) and a
512-token output slice (4 x 128-token quarters, one per 1024-token span).

Key design points (vs v1):
  - Global slot schedule: one 128-key score chunk per slot; ctx matmuls
    lag scores by 2 slots (h0) / 3 slots (h1); exp (ACT engine) is one
    slot behind scores with double-buffered PSUM so the PE never stalls
    on the activation engine and keeps its fast p-state.
  - QKV projections for batch 1 and the output projection run as filler
    matmul units inside attention slots that would otherwise idle.
  - Four quarter AllToAlls (256KB each) replace the AllGather (7MB/core):
    each core receives exactly the full-width ctx^T for its own tokens,
    so phase 3 is rank-static and overlaps with attention.

Host passes activations/weights pre-transposed and pre-cast to bf16
(feature-major), so the device spends no time on casts/transposes.
"""

import os
import sys

for _p in ("/opt/trn_rl_repo", "/root/.axon_site/_ro/trn_rl_repo"):
    if os.path.isdir(_p) and _p not in sys.path:
        sys.path.append(_p)

import ml_dtypes
import numpy as np

# Shim antenv.axon_hooks (absent in some images): bass_utils imports it
# unconditionally when tracing is requested via env.
try:
    import antenv.axon_hooks  # noqa: F401
except Exception:
    import types as _types
    try:
        import antenv as _antenv
        _m = _types.ModuleType("antenv.axon_hooks")
        _m._hook = None
        _m.set_axon_ntff_profile_hook = lambda h, _m=_m: setattr(_m, "_hook", h)
        _m.get_axon_ntff_profile_hook = lambda _m=_m: _m._hook
        sys.modules["antenv.axon_hooks"] = _m
        _antenv.axon_hooks = _m
    except Exception:
        pass

import concourse.bass as bass  # noqa: F401
import concourse.tile as tile
from concourse import bacc, mybir
from concourse.bass_utils import run_bass_kernel_spmd

F32 = mybir.dt.float32
BF16 = mybir.dt.bfloat16
BF16_NP = ml_dtypes.bfloat16

NCORES = 8
H = 16   # heads total
DH = 64  # head dim
LN_EPS = 1e-12


def build_bert_kernel(S=2048, B=2, D=1024):
    P = 128
    NTOK = S * B              # 4096 batch-major tokens
    TPC = NTOK // NCORES      # 512 output tokens per core (4 quarters)
    CCH = D // P              # 8 contraction chunks
    HPC = H // NCORES         # 2 heads per core
    DL = HPC * DH             # 128 local feature dims
    NJ = S // P               # 16 key chunks per batch
    NI = S // 512             # 4 query blocks per batch
    NSLOT = B * NI * NJ       # 128 score chunk-slots
    NQTR = NTOK // 1024       # 4 quarters
    NVT = NTOK // P           # 32 v token tiles

    nc = bacc.Bacc("TRN2", target_bir_lowering=False, debug=False,
                   num_devices=NCORES)

    def din(name, shape, dt=F32):
        return nc.dram_tensor(name, list(shape), dt, kind="ExternalInput").ap()

    xqT = din("xqT", (D, NTOK), BF16)
    xkT = din("xkT", (D, NTOK), BF16)
    xvT = din("xvT", (D, NTOK), BF16)
    wqT = din("wqT", (D, DL), BF16)
    wkT = din("wkT", (D, DL), BF16)
    wvT = din("wvT", (D, DL), BF16)
    woT = din("woT", (D, D), BF16)
    bq = din("bq", (DL, 1))
    bk = din("bk", (DL, 1))
    bv = din("bv", (1, DL))
    bo = din("bo", (1, D))
    lnw = din("lnw", (1, D))
    lnb = din("lnb", (1, D))
    resid = din("resid", (TPC, D))
    out = nc.dram_tensor("out", [TPC, D], F32, kind="ExternalOutput").ap()

    # per-quarter exchange buffers: piece p of a2a[q] = ctx^T of this
    # core's heads for tokens [q*1024 + p*128, +128) -> destined core p.
    a2a = [nc.dram_tensor(f"a2a{q}", [NCORES, P, P], BF16).ap()
           for q in range(NQTR)]
    ag = [nc.dram_tensor(f"ag{q}", [NCORES, P, P], BF16).ap()
          for q in range(NQTR)]
    GRP = [list(range(NCORES))]

    with tile.TileContext(nc) as tc:
        with (
            tc.tile_pool(name="persist", bufs=1) as persist,
            tc.tile_pool(name="small", bufs=1) as small,
            tc.tile_pool(name="xT", bufs=1) as xt_pool,
            tc.tile_pool(name="work", bufs=1) as work,
            tc.tile_pool(name="ps_sc", bufs=1, space="PSUM") as ps_sc,
            tc.tile_pool(name="ps_cps", bufs=1, space="PSUM") as ps_cps,
        ):
            SCB = 3  # scores/filler PSUM ring depth (3 x 2 banks)

            def sc_tile():
                return ps_sc.tile([P, 2 * 512], F32, name="sc",
                                  tag="sc", bufs=SCB)
            # ---- weights into SBUF (woT deferred into the slot loop) ----
            wqT_sb = persist.tile([P, CCH, DL], BF16)
            wkT_sb = persist.tile([P, CCH, DL], BF16)
            wvT_sb = persist.tile([P, CCH, DL], BF16)
            for wi, (w_d, w_sb) in enumerate(
                    ((wkT, wkT_sb), (wqT, wqT_sb), (wvT, wvT_sb))):
                for c in range(CCH):
                    (nc.sync if (wi + c) % 2 else nc.gpsimd).dma_start(
                        out=w_sb[:, c, :], in_=w_d[c * P:(c + 1) * P, :])
            woT_sb = persist.tile([P, CCH, D], BF16)

            def load_woT():
                for c in range(CCH):
                    nc.sync.dma_start(out=woT_sb[:, c, :],
                                      in_=woT[c * P:(c + 1) * P, :])

            # ---- constant / bias tiles ----
            bq_sb = small.tile([P, 1], F32)
            bk_sb = small.tile([P, 1], F32)
            nc.sync.dma_start(out=bq_sb, in_=bq)
            nc.sync.dma_start(out=bk_sb, in_=bk)
            bv_bc = small.tile([P, DL], F32)
            nc.gpsimd.dma_start(out=bv_bc, in_=bv.to_broadcast((P, DL)))
            bo_bc = small.tile([P, D], F32)
            nc.gpsimd.dma_start(out=bo_bc, in_=bo.to_broadcast((P, D)))
            lnw_bc = small.tile([P, D], F32)
            nc.gpsimd.dma_start(out=lnw_bc, in_=lnw.to_broadcast((P, D)))
            lnb_bc = small.tile([P, D], F32)
            nc.gpsimd.dma_start(out=lnb_bc, in_=lnb.to_broadcast((P, D)))
            eps_sb = small.tile([P, 1], F32)
            nc.vector.memset(eps_sb, LN_EPS)

            # ---- persistent activation buffers ----
            qT_sb = persist.tile([P, NTOK], BF16)   # [dloc, tok]
            kT_sb = persist.tile([P, NTOK], BF16)
            v_sb = persist.tile([P, NVT, HPC * (DH + 1)], BF16)
            nc.vector.memset(v_sb[:, :, DH:DH + 1], 1.0)
            nc.vector.memset(v_sb[:, :, 2 * DH + 1:2 * DH + 2], 1.0)
            ERD = 8
            e_ring = persist.tile([P, ERD, 2 * 512], BF16)  # exp ring
            ctxF = [persist.tile([P, CCH, P], BF16, name=f"ctxF{q}")
                    for q in range(NQTR)]
            y_sb = [persist.tile([P, D], F32, name=f"y{q}")
                    for q in range(NQTR)]

            # ---- x tiles: 512-token granules, loaded just-in-time in
            # consumption order across 3 DMA queues ----
            xt = {}
            ei = 0
            dmaq = (nc.sync, nc.scalar, nc.gpsimd)
            xd = {0: xqT, 1: xkT, 2: xvT}
            LOAD_ORDER = [(1, 0), (0, 0), (2, 0), (1, 1), (2, 1), (1, 2),
                          (2, 2), (1, 3), (2, 3), (0, 1), (0, 2), (0, 3)]
            for b in range(B):
                for ti, n in LOAD_ORDER:
                    for c in range(CCH):
                        t = xt_pool.tile([P, 512], BF16,
                                         name=f"xT{ti}_{c}_{b}_{n}",
                                         tag="xT", bufs=48)
                        xt[(ti, c, b, n)] = t
                        dmaq[ei % 3].dma_start(
                            out=t, in_=xd[ti][c * P:(c + 1) * P,
                                             b * S + n * 512:
                                             b * S + (n + 1) * 512])
                        ei += 1

            # ================= unit builders =================
            def qk_unit(ti, b, n):
                # projection of 512 tokens onto this core's 128 q/k dims
                w_sb, b_sb, o_sb = ((wqT_sb, bq_sb, qT_sb),
                                    (wkT_sb, bk_sb, kT_sb))[ti]
                ps = sc_tile()[:, 0:512]
                for c in range(CCH):
                    nc.tensor.matmul(ps, w_sb[:, c, :], xt[(ti, c, b, n)],
                                     start=(c == 0), stop=(c == CCH - 1))
                tok0 = b * S + n * 512
                nc.vector.tensor_scalar_add(o_sb[:, tok0:tok0 + 512], ps, b_sb)

            def v_unit(b, nt):
                # v projection for 4 token tiles (512 tokens)
                ps = sc_tile()[:, 0:512]
                for k in range(4):
                    for c in range(CCH):
                        nc.tensor.matmul(
                            ps[:, k * P:(k + 1) * P],
                            xt[(2, c, b, nt)][:, k * P:(k + 1) * P],
                            wvT_sb[:, c, :],
                            start=(c == 0), stop=(c == CCH - 1))
                for k in range(4):
                    it = b * (S // P) + nt * 4 + k
                    dst = v_sb[:, it, :].rearrange(
                        "p (h x) -> p h x", h=HPC)[:, :, 0:DH]
                    src = ps[:, k * P:(k + 1) * P].rearrange(
                        "p (h x) -> p h x", h=HPC)
                    bvr = bv_bc.rearrange("p (h x) -> p h x", h=HPC)
                    nc.vector.tensor_add(dst, src, bvr)

            resid_sb = {}

            def op_unit(i2, n):
                # output projection: 128 tokens x 512 output dims
                cF = ctxF[i2]
                ps = sc_tile()[:, 0:512]
                for c in range(CCH):
                    nc.tensor.matmul(ps, cF[:, c, :],
                                     woT_sb[:, c, n * 512:(n + 1) * 512],
                                     start=(c == 0), stop=(c == CCH - 1))
                if n == 0:
                    rt = work.tile([P, D], F32, tag="resid", bufs=2,
                                   name=f"resid{i2}")
                    resid_sb[i2] = rt
                    nc.sync.dma_start(out=rt,
                                      in_=resid[i2 * P:(i2 + 1) * P, :])
                sl = slice(n * 512, (n + 1) * 512)
                y = y_sb[i2]
                nc.vector.tensor_add(y[:, sl], ps, bo_bc[:, sl])
                nc.vector.tensor_add(y[:, sl], y[:, sl], resid_sb[i2][:, sl])

            def ln_unit(i2):
                y = y_sb[i2]
                y3 = y.rearrange("p (g d) -> p g d", g=2)
                stats = work.tile([P, 2, 6], F32, tag="stats", bufs=2)
                for g in range(2):
                    nc.vector.bn_stats(out=stats[:, g, :], in_=y3[:, g, :])
                mv = work.tile([P, 2], F32, tag="mv", bufs=2)
                nc.vector.bn_aggr(out=mv, in_=stats)
                std = work.tile([P, 1], F32, tag="std", bufs=2)
                nc.scalar.activation(std, mv[:, 1:2],
                                     mybir.ActivationFunctionType.Sqrt,
                                     bias=eps_sb)
                rstd = work.tile([P, 1], F32, tag="rstd", bufs=2)
                nc.vector.reciprocal(rstd, std)
                t32 = work.tile([P, D], F32, tag="t32", bufs=2)
                nc.vector.tensor_scalar(
                    out=t32, in0=y, scalar1=mv[:, 0:1], scalar2=rstd,
                    op0=mybir.AluOpType.subtract, op1=mybir.AluOpType.mult)
                of = work.tile([P, D], F32, tag="of", bufs=2)
                nc.vector.tensor_mul(of, t32, lnw_bc)
                nc.vector.tensor_add(of, of, lnb_bc)
                nc.sync.dma_start(out=out[i2 * P:(i2 + 1) * P, :], in_=of)

            # ---- filler placement: slot -> list of unit closures ----
            # units are placed just after their input DMAs can have
            # landed and just before their outputs are first consumed.
            fillers = {}

            def add_filler(s, fn):
                fillers.setdefault(s, []).append(fn)

            b0_rest = [(0, lambda: v_unit(0, 0)),
                       (2, lambda: qk_unit(1, 0, 1)),
                       (4, lambda: v_unit(0, 1)),
                       (6, lambda: qk_unit(1, 0, 2)),
                       (8, lambda: v_unit(0, 2)),
                       (10, lambda: qk_unit(1, 0, 3)),
                       (12, lambda: v_unit(0, 3)),
                       (14, lambda: qk_unit(0, 0, 1)),
                       (24, lambda: qk_unit(0, 0, 2)),
                       (40, lambda: qk_unit(0, 0, 3))]
            b1_units = [(26, lambda: qk_unit(1, 1, 0)),
                        (28, lambda: qk_unit(0, 1, 0)),
                        (30, lambda: qk_unit(1, 1, 1)),
                        (32, lambda: v_unit(1, 0)),
                        (34, lambda: qk_unit(1, 1, 2)),
                        (36, lambda: v_unit(1, 1)),
                        (38, lambda: qk_unit(1, 1, 3)),
                        (42, lambda: v_unit(1, 2)),
                        (44, lambda: v_unit(1, 3)),
                        (46, lambda: qk_unit(0, 1, 1)),
                        (48, lambda: qk_unit(0, 1, 2)),
                        (50, lambda: qk_unit(0, 1, 3))]
            for s, fn in b0_rest + b1_units:
                add_filler(s, fn)
            add_filler(41, load_woT)
            add_filler(52, lambda: op_unit(0, 0))
            add_filler(56, lambda: op_unit(0, 1))
            add_filler(72, lambda: op_unit(1, 0))
            add_filler(76, lambda: op_unit(1, 1))
            add_filler(102, lambda: op_unit(2, 0))
            add_filler(106, lambda: op_unit(2, 1))
            add_filler(100, lambda: ln_unit(0))
            add_filler(100, lambda: ln_unit(1))

            # ================= main slot loop =================
            cps = [None, None]        # live ctx PSUM tile per head
            ctxo = {}                 # (b, i) -> staging tile

            def scores_step(s):
                b, r = divmod(s, NI * NJ)
                i, j = divmod(r, NJ)
                sc = sc_tile()
                jc0 = b * S + j * P
                ic0 = b * S + i * 512
                for h in range(HPC):
                    nc.tensor.matmul(
                        sc[:, h * 512:(h + 1) * 512],
                        kT_sb[h * DH:(h + 1) * DH, jc0:jc0 + P],
                        qT_sb[h * DH:(h + 1) * DH, ic0:ic0 + 512])
                nc.scalar.activation(e_ring[:, s % ERD, :], sc,
                                     mybir.ActivationFunctionType.Exp)

            def ctx_step(c, h):
                b, r = divmod(c, NI * NJ)
                i, j = divmod(r, NJ)
                vt = b * NJ + j
                if j == 0:
                    cps[h] = ps_cps.tile([DH + 1, 512], F32, name=f"cps{h}",
                                         tag=f"cps{h}", bufs=1)
                nc.tensor.matmul(
                    cps[h],
                    v_sb[:, vt, h * (DH + 1):(h + 1) * (DH + 1)],
                    e_ring[:, c % ERD, h * 512:(h + 1) * 512],
                    start=(j == 0), stop=(j == NJ - 1))
                if j != NJ - 1:
                    return
                # ---- softmax normalize + stage for exchange ----
                # single fast PSUM->SBUF copy frees the cps bank; the
                # normalize chain then runs off the PE critical path
                if h == 0:
                    ctxo[(b, i)] = work.tile([P, 512], BF16, tag="ctxo",
                                             bufs=2, name=f"ctxo{b}_{i}")
                co = ctxo[(b, i)]
                cs = work.tile([DH, 512], F32, tag="cs", bufs=2)
                nc.vector.tensor_copy(cs, cps[h][0:DH, :])
                ssum = work.tile([1, 512], F32, tag="ssum", bufs=2)
                nc.vector.tensor_copy(ssum, cps[h][DH:DH + 1, :])
                rcp = work.tile([1, 512], F32, tag="rcp", bufs=2)
                nc.vector.reciprocal_approx_fast(rcp, ssum)
                rbc = work.tile([DH, 512], F32, tag="rbc", bufs=2)
                nc.gpsimd.partition_broadcast(rbc, rcp)
                nc.vector.tensor_mul(co[h * DH:(h + 1) * DH, :],
                                     cs, rbc)
                if h == HPC - 1:
                    tok0 = b * S + i * 512
                    q = tok0 // 1024
                    p0 = (tok0 % 1024) // P
                    for t4 in range(4):
                        nc.sync.dma_start(
                            out=a2a[q][p0 + t4, :, :],
                            in_=co[:, t4 * P:(t4 + 1) * P])
                    if i % 2 == 1:  # quarter complete -> exchange
                        nc.gpsimd.collective_compute(
                            "AllToAll", mybir.AluOpType.bypass,
                            replica_groups=GRP,
                            ins=[a2a[q].opt()], outs=[ag[q].opt()])
                        for cc in range(CCH):
                            nc.sync.dma_start(out=ctxF[q][:, cc, :],
                                              in_=ag[q][cc, :, :])

            # prologue: just enough QKV(b0) for the first score block;
            # the rest streams in as fillers behind the DMAs
            qk_unit(1, 0, 0)
            qk_unit(0, 0, 0)

            for s in range(NSLOT + 3):
                if s < NSLOT:
                    scores_step(s)
                if 0 <= s - 2 < NSLOT:
                    ctx_step(s - 2, 0)
                if 0 <= s - 3 < NSLOT:
                    ctx_step(s - 3, 1)
                for fn in fillers.get(s, ()):
                    fn()

            # epilogue: LN(2) first (overlaps the last AllToAll), then
            # the last quarter's projection + its LayerNorm
            ln_unit(2)
            op_unit(3, 0)
            op_unit(3, 1)
            ln_unit(3)

    nc.compile()
    return nc


_NC_CACHE = {}


def _get_nc(S=2048, B=2, D=1024):
    key = (S, B, D)
    if key not in _NC_CACHE:
        _NC_CACHE[key] = build_bert_kernel(S, B, D)
    return _NC_CACHE[key]


def make_in_maps(query_tensor, key_tensor, value_tensor, Wq, bq, Wk, bk,
                 Wv, bv, Wo, bo, ln_w, ln_b):
    S, B, D = query_tensor.shape
    NTOK = S * B
    DL = (H // NCORES) * DH

    def bm(x):  # (S, B, D) -> batch-major (B*S, D) float32
        return np.ascontiguousarray(
            np.asarray(x, np.float32).transpose(1, 0, 2).reshape(NTOK, D))

    def bmT(x):  # feature-major bf16 (D, B*S)
        return np.ascontiguousarray(bm(x).T.astype(BF16_NP))

    xq = bm(query_tensor)
    xqT, xkT, xvT = bmT(query_tensor), bmT(key_tensor), bmT(value_tensor)
    woT = np.ascontiguousarray(
        np.asarray(Wo, np.float32).T.astype(BF16_NP))
    f32 = lambda a: np.ascontiguousarray(np.asarray(a, np.float32))
    bf16T = lambda a: np.ascontiguousarray(
        np.asarray(a, np.float32).T.astype(BF16_NP))
    in_maps = []
    for c in range(NCORES):
        sl = slice(c * DL, (c + 1) * DL)
        rs = np.concatenate([xq[q * 1024 + c * 128:q * 1024 + (c + 1) * 128]
                             for q in range(NTOK // 1024)], axis=0)
        in_maps.append({
            "xqT": xqT, "xkT": xkT, "xvT": xvT,
            "wqT": bf16T(Wq[sl]), "wkT": bf16T(Wk[sl]),
            "wvT": bf16T(Wv[sl]), "woT": woT,
            "bq": f32(bq[sl]).reshape(DL, 1),
            "bk": f32(bk[sl]).reshape(DL, 1),
            "bv": f32(bv[sl]).reshape(1, DL),
            "bo": f32(bo).reshape(1, D),
            "lnw": f32(ln_w).reshape(1, D),
            "lnb": f32(ln_b).reshape(1, D),
            "resid": np.ascontiguousarray(rs),
        })
    return in_maps


def assemble_output(results, S, B, D):
    NTOK = S * B
    full = np.empty((NTOK, D), np.float32)
    for c, r in enumerate(results):
        o = r["out"]  # (512, D): 4 x 128-token quarter slices
        for q in range(NTOK // 1024):
            full[q * 1024 + c * 128:q * 1024 + (c + 1) * 128] = \
                o[q * 128:(q + 1) * 128]
    return np.ascontiguousarray(
        full.reshape(B, S, D).transpose(1, 0, 2))


def kernel(**inputs):
    S, B, D = inputs["query_tensor"].shape
    nc = _get_nc(S, B, D)
    in_maps = make_in_maps(**inputs)
    res = run_bass_kernel_spmd(nc, in_maps, list(range(NCORES)))
    return assemble_output(res.results, S, B, D)


# revision 46
# speedup vs baseline: 1.0297x; 1.0025x over previous
"""BertAttention Trainium2 kernel — 8-core SPMD, v2.

Sharding: each core owns 2 heads (128 of the 1024 feature dims) and a
512-token output slice (4 x 128-token quarters, one per 1024-token span).

Key design points (vs v1):
  - Global slot schedule: one 128-key score chunk per slot; ctx matmuls
    lag scores by 2 slots (h0) / 3 slots (h1); exp (ACT engine) is one
    slot behind scores with double-buffered PSUM so the PE never stalls
    on the activation engine and keeps its fast p-state.
  - QKV projections for batch 1 and the output projection run as filler
    matmul units inside attention slots that would otherwise idle.
  - Four quarter AllToAlls (256KB each) replace the AllGather (7MB/core):
    each core receives exactly the full-width ctx^T for its own tokens,
    so phase 3 is rank-static and overlaps with attention.

Host passes activations/weights pre-transposed and pre-cast to bf16
(feature-major), so the device spends no time on casts/transposes.
"""

import os
import sys

for _p in ("/opt/trn_rl_repo", "/root/.axon_site/_ro/trn_rl_repo"):
    if os.path.isdir(_p) and _p not in sys.path:
        sys.path.append(_p)

import ml_dtypes
import numpy as np

# Shim antenv.axon_hooks (absent in some images): bass_utils imports it
# unconditionally when tracing is requested via env.
try:
    import antenv.axon_hooks  # noqa: F401
except Exception:
    import types as _types
    try:
        import antenv as _antenv
        _m = _types.ModuleType("antenv.axon_hooks")
        _m._hook = None
        _m.set_axon_ntff_profile_hook = lambda h, _m=_m: setattr(_m, "_hook", h)
        _m.get_axon_ntff_profile_hook = lambda _m=_m: _m._hook
        sys.modules["antenv.axon_hooks"] = _m
        _antenv.axon_hooks = _m
    except Exception:
        pass

import concourse.bass as bass  # noqa: F401
import concourse.tile as tile
from concourse import bacc, mybir
from concourse.bass_utils import run_bass_kernel_spmd

F32 = mybir.dt.float32
BF16 = mybir.dt.bfloat16
BF16_NP = ml_dtypes.bfloat16

NCORES = 8
H = 16   # heads total
DH = 64  # head dim
LN_EPS = 1e-12


def build_bert_kernel(S=2048, B=2, D=1024):
    P = 128
    NTOK = S * B              # 4096 batch-major tokens
    TPC = NTOK // NCORES      # 512 output tokens per core (4 quarters)
    CCH = D // P              # 8 contraction chunks
    HPC = H // NCORES         # 2 heads per core
    DL = HPC * DH             # 128 local feature dims
    NJ = S // P               # 16 key chunks per batch
    NI = S // 512             # 4 query blocks per batch
    NSLOT = B * NI * NJ       # 128 score chunk-slots
    NQTR = NTOK // 1024       # 4 quarters
    NVT = NTOK // P           # 32 v token tiles

    nc = bacc.Bacc("TRN2", target_bir_lowering=False, debug=False,
                   num_devices=NCORES)

    def din(name, shape, dt=F32):
        return nc.dram_tensor(name, list(shape), dt, kind="ExternalInput").ap()

    # x inputs come host-tiled: [CCH, B, 128, S] so each (chunk, batch)
    # tile is one fully-contiguous 512KB DMA
    xqT = din("xqT", (CCH, B, P, S), BF16)
    xkT = din("xkT", (CCH, B, P, S), BF16)
    xvT = din("xvT", (CCH, B, P, S), BF16)
    wqT = din("wqT", (D, DL), BF16)
    wkT = din("wkT", (D, DL), BF16)
    wvT = din("wvT", (D, DL), BF16)
    woT = din("woT", (CCH, P, D), BF16)
    bq = din("bq", (DL, 1))
    bk = din("bk", (DL, 1))
    bv = din("bv", (1, DL))
    bo = din("bo", (1, D))
    lnw = din("lnw", (1, D))
    lnb = din("lnb", (1, D))
    resid = din("resid", (TPC, D))
    out = nc.dram_tensor("out", [TPC, D], F32, kind="ExternalOutput").ap()

    # per-quarter exchange buffers: piece p of a2a[q] = ctx^T of this
    # core's heads for tokens [q*1024 + p*128, +128) -> destined core p.
    a2a = [nc.dram_tensor(f"a2a{q}", [NCORES, P, P], BF16).ap()
           for q in range(NQTR)]
    ag = [nc.dram_tensor(f"ag{q}", [NCORES, P, P], BF16).ap()
          for q in range(NQTR)]
    GRP = [list(range(NCORES))]

    with tile.TileContext(nc) as tc:
        with (
            tc.tile_pool(name="persist", bufs=1) as persist,
            tc.tile_pool(name="small", bufs=1) as small,
            tc.tile_pool(name="xT", bufs=1) as xt_pool,
            tc.tile_pool(name="work", bufs=1) as work,
            tc.tile_pool(name="ps_sc", bufs=1, space="PSUM") as ps_sc,
            tc.tile_pool(name="ps_cps", bufs=1, space="PSUM") as ps_cps,
        ):
            SCB = 3  # scores/filler PSUM ring depth (3 x 2 banks)

            def sc_tile():
                return ps_sc.tile([P, 2 * 512], F32, name="sc",
                                  tag="sc", bufs=SCB)
            # ---- weights into SBUF (woT deferred into the slot loop) ----
            wqT_sb = persist.tile([P, CCH, DL], BF16)
            wkT_sb = persist.tile([P, CCH, DL], BF16)
            wvT_sb = persist.tile([P, CCH, DL], BF16)
            for wi, (w_d, w_sb) in enumerate(
                    ((wkT, wkT_sb), (wqT, wqT_sb), (wvT, wvT_sb))):
                for c in range(CCH):
                    (nc.sync if (wi + c) % 2 else nc.gpsimd).dma_start(
                        out=w_sb[:, c, :], in_=w_d[c * P:(c + 1) * P, :])
            woT_sb = persist.tile([P, CCH, D], BF16)

            def load_woT():
                for c in range(CCH):
                    nc.sync.dma_start(out=woT_sb[:, c, :], in_=woT[c, :, :])

            # ---- constant / bias tiles ----
            bq_sb = small.tile([P, 1], F32)
            bk_sb = small.tile([P, 1], F32)
            nc.sync.dma_start(out=bq_sb, in_=bq)
            nc.sync.dma_start(out=bk_sb, in_=bk)
            bv_bc = small.tile([P, DL], F32)
            nc.gpsimd.dma_start(out=bv_bc, in_=bv.to_broadcast((P, DL)))
            bo_bc = small.tile([P, D], F32)
            nc.gpsimd.dma_start(out=bo_bc, in_=bo.to_broadcast((P, D)))
            lnw_bc = small.tile([P, D], F32)
            nc.gpsimd.dma_start(out=lnw_bc, in_=lnw.to_broadcast((P, D)))
            lnb_bc = small.tile([P, D], F32)
            nc.gpsimd.dma_start(out=lnb_bc, in_=lnb.to_broadcast((P, D)))
            eps_sb = small.tile([P, 1], F32)
            nc.vector.memset(eps_sb, LN_EPS)

            # ---- persistent activation buffers ----
            qT_sb = persist.tile([P, NTOK], BF16)   # [dloc, tok]
            kT_sb = persist.tile([P, NTOK], BF16)
            v_sb = persist.tile([P, NVT, HPC * (DH + 1)], BF16)
            nc.vector.memset(v_sb[:, :, DH:DH + 1], 1.0)
            nc.vector.memset(v_sb[:, :, 2 * DH + 1:2 * DH + 2], 1.0)
            ERD = 8
            e_ring = persist.tile([P, ERD, 2 * 512], BF16)  # exp ring
            ctxF = [persist.tile([P, CCH, P], BF16, name=f"ctxF{q}")
                    for q in range(NQTR)]
            y_sb = [persist.tile([P, D], F32, name=f"y{q}")
                    for q in range(NQTR)]

            # ---- x tiles: one contiguous 512KB DMA per (proj, chunk,
            # batch) in prologue consumption order (k, q, v; b0 then b1).
            # Ring-gated b1 issues resolve against prologue-consumed b0
            # buffers, so no sequencer ever blocks against in-loop work.
            xt = {}
            ei = 0
            dmaq = (nc.sync, nc.scalar, nc.gpsimd)
            xd = {0: xqT, 1: xkT, 2: xvT}
            for b in range(B):
                for ti in (1, 0, 2):        # k, q, v
                    for c in range(CCH):
                        t = xt_pool.tile([P, S], BF16,
                                         name=f"xT{ti}_{c}_{b}",
                                         tag="xT", bufs=16)
                        xt[(ti, c, b)] = t
                        dmaq[ei % 3].dma_start(out=t, in_=xd[ti][c, b, :, :])
                        ei += 1

            # ================= unit builders =================
            def qk_unit(ti, b, n):
                # projection of 512 tokens onto this core's 128 q/k dims
                w_sb, b_sb, o_sb = ((wqT_sb, bq_sb, qT_sb),
                                    (wkT_sb, bk_sb, kT_sb))[ti]
                ps = sc_tile()[:, 0:512]
                for c in range(CCH):
                    nc.tensor.matmul(ps, w_sb[:, c, :],
                                     xt[(ti, c, b)][:, n * 512:(n + 1) * 512],
                                     start=(c == 0), stop=(c == CCH - 1))
                tok0 = b * S + n * 512
                nc.vector.tensor_scalar_add(o_sb[:, tok0:tok0 + 512], ps, b_sb)

            def v_unit(b, nt):
                # v projection for 4 token tiles (512 tokens)
                ps = sc_tile()[:, 0:512]
                for k in range(4):
                    itl = nt * 4 + k
                    for c in range(CCH):
                        nc.tensor.matmul(
                            ps[:, k * P:(k + 1) * P],
                            xt[(2, c, b)][:, itl * P:(itl + 1) * P],
                            wvT_sb[:, c, :],
                            start=(c == 0), stop=(c == CCH - 1))
                for k in range(4):
                    it = b * (S // P) + nt * 4 + k
                    dst = v_sb[:, it, :].rearrange(
                        "p (h x) -> p h x", h=HPC)[:, :, 0:DH]
                    src = ps[:, k * P:(k + 1) * P].rearrange(
                        "p (h x) -> p h x", h=HPC)
                    bvr = bv_bc.rearrange("p (h x) -> p h x", h=HPC)
                    nc.vector.tensor_add(dst, src, bvr)

            resid_sb = {}

            def op_unit(i2, n):
                # output projection: 128 tokens x 512 output dims
                cF = ctxF[i2]
                ps = sc_tile()[:, 0:512]
                for c in range(CCH):
                    nc.tensor.matmul(ps, cF[:, c, :],
                                     woT_sb[:, c, n * 512:(n + 1) * 512],
                                     start=(c == 0), stop=(c == CCH - 1))
                if n == 0:
                    rt = work.tile([P, D], F32, tag="resid", bufs=2,
                                   name=f"resid{i2}")
                    resid_sb[i2] = rt
                    nc.sync.dma_start(out=rt,
                                      in_=resid[i2 * P:(i2 + 1) * P, :])
                sl = slice(n * 512, (n + 1) * 512)
                y = y_sb[i2]
                nc.vector.tensor_add(y[:, sl], ps, bo_bc[:, sl])
                nc.vector.tensor_add(y[:, sl], y[:, sl], resid_sb[i2][:, sl])

            def ln_unit(i2):
                y = y_sb[i2]
                y3 = y.rearrange("p (g d) -> p g d", g=2)
                stats = work.tile([P, 2, 6], F32, tag="stats", bufs=2)
                for g in range(2):
                    nc.vector.bn_stats(out=stats[:, g, :], in_=y3[:, g, :])
                mv = work.tile([P, 2], F32, tag="mv", bufs=2)
                nc.vector.bn_aggr(out=mv, in_=stats)
                std = work.tile([P, 1], F32, tag="std", bufs=2)
                nc.scalar.activation(std, mv[:, 1:2],
                                     mybir.ActivationFunctionType.Sqrt,
                                     bias=eps_sb)
                rstd = work.tile([P, 1], F32, tag="rstd", bufs=2)
                nc.vector.reciprocal(rstd, std)
                t32 = work.tile([P, D], F32, tag="t32", bufs=1)
                nc.vector.tensor_scalar(
                    out=t32, in0=y, scalar1=mv[:, 0:1], scalar2=rstd,
                    op0=mybir.AluOpType.subtract, op1=mybir.AluOpType.mult)
                of = work.tile([P, D], F32, tag="of", bufs=2)
                nc.vector.tensor_mul(of, t32, lnw_bc)
                nc.vector.tensor_add(of, of, lnb_bc)
                nc.sync.dma_start(out=out[i2 * P:(i2 + 1) * P, :], in_=of)

            # ---- filler placement: slot -> list of unit closures ----
            # units are placed just after their input DMAs can have
            # landed and just before their outputs are first consumed.
            fillers = {}

            def add_filler(s, fn):
                fillers.setdefault(s, []).append(fn)

            # b1 fillers: placed after their x DMAs can have landed
            # (b1 loads flow once the prologue frees the xt ring)
            sched = [(20, load_woT),
                     (26, lambda: qk_unit(1, 1, 0)),
                     (28, lambda: qk_unit(1, 1, 1)),
                     (30, lambda: qk_unit(1, 1, 2)),
                     (32, lambda: qk_unit(1, 1, 3)),
                     (34, lambda: qk_unit(0, 1, 0)),
                     (36, lambda: qk_unit(0, 1, 1)),
                     (38, lambda: qk_unit(0, 1, 2)),
                     (40, lambda: qk_unit(0, 1, 3)),
                     (44, lambda: v_unit(1, 0)),
                     (46, lambda: v_unit(1, 1)),
                     (48, lambda: v_unit(1, 2)),
                     (50, lambda: v_unit(1, 3))]
            for s, fn in sched:
                add_filler(s, fn)
            add_filler(54, lambda: op_unit(0, 0))
            add_filler(58, lambda: op_unit(0, 1))
            add_filler(72, lambda: op_unit(1, 0))
            add_filler(76, lambda: op_unit(1, 1))
            add_filler(102, lambda: op_unit(2, 0))
            add_filler(106, lambda: op_unit(2, 1))
            add_filler(100, lambda: ln_unit(0))
            add_filler(100, lambda: ln_unit(1))

            # ================= main slot loop =================
            cps = [None, None]        # live ctx PSUM tile per head
            ctxo = {}                 # (b, i) -> staging tile

            def scores_step(s):
                b, r = divmod(s, NI * NJ)
                i, j = divmod(r, NJ)
                sc = sc_tile()
                jc0 = b * S + j * P
                ic0 = b * S + i * 512
                for h in range(HPC):
                    nc.tensor.matmul(
                        sc[:, h * 512:(h + 1) * 512],
                        kT_sb[h * DH:(h + 1) * DH, jc0:jc0 + P],
                        qT_sb[h * DH:(h + 1) * DH, ic0:ic0 + 512])
                nc.scalar.activation(e_ring[:, s % ERD, :], sc,
                                     mybir.ActivationFunctionType.Exp)

            def ctx_step(c, h):
                b, r = divmod(c, NI * NJ)
                i, j = divmod(r, NJ)
                vt = b * NJ + j
                if j == 0:
                    cps[h] = ps_cps.tile([DH + 1, 512], F32, name=f"cps{h}",
                                         tag=f"cps{h}", bufs=1)
                nc.tensor.matmul(
                    cps[h],
                    v_sb[:, vt, h * (DH + 1):(h + 1) * (DH + 1)],
                    e_ring[:, c % ERD, h * 512:(h + 1) * 512],
                    start=(j == 0), stop=(j == NJ - 1))
                if j != NJ - 1:
                    return
                # ---- softmax normalize + stage for exchange ----
                # single fast PSUM->SBUF copy frees the cps bank; the
                # normalize chain then runs off the PE critical path
                if h == 0:
                    ctxo[(b, i)] = work.tile([P, 512], BF16, tag="ctxo",
                                             bufs=2, name=f"ctxo{b}_{i}")
                co = ctxo[(b, i)]
                cs = work.tile([DH, 512], F32, tag="cs", bufs=2)
                nc.vector.tensor_copy(cs, cps[h][0:DH, :])
                ssum = work.tile([1, 512], F32, tag="ssum", bufs=2)
                nc.vector.tensor_copy(ssum, cps[h][DH:DH + 1, :])
                rcp = work.tile([1, 512], F32, tag="rcp", bufs=2)
                nc.vector.reciprocal_approx_fast(rcp, ssum)
                rbc = work.tile([DH, 512], F32, tag="rbc", bufs=2)
                nc.gpsimd.partition_broadcast(rbc, rcp)
                nc.vector.tensor_mul(co[h * DH:(h + 1) * DH, :],
                                     cs, rbc)
                if h == HPC - 1:
                    tok0 = b * S + i * 512
                    q = tok0 // 1024
                    p0 = (tok0 % 1024) // P
                    for t4 in range(4):
                        nc.gpsimd.dma_start(
                            out=a2a[q][p0 + t4, :, :],
                            in_=co[:, t4 * P:(t4 + 1) * P])
                    if i % 2 == 1:  # quarter complete -> exchange
                        nc.gpsimd.collective_compute(
                            "AllToAll", mybir.AluOpType.bypass,
                            replica_groups=GRP,
                            ins=[a2a[q].opt()], outs=[ag[q].opt()])
                        for cc in range(CCH):
                            nc.sync.dma_start(out=ctxF[q][:, cc, :],
                                              in_=ag[q][cc, :, :])

            # prologue: full QKV(b0) (solid PE block, no ring hazards)
            for n in range(4):
                qk_unit(1, 0, n)
            for n in range(4):
                qk_unit(0, 0, n)
            for nt in range(4):
                v_unit(0, nt)

            for s in range(NSLOT + 3):
                if s < NSLOT:
                    scores_step(s)
                if 0 <= s - 2 < NSLOT:
                    ctx_step(s - 2, 0)
                if 0 <= s - 3 < NSLOT:
                    ctx_step(s - 3, 1)
                for fn in fillers.get(s, ()):
                    fn()

            # epilogue: LN(2) first (overlaps the last AllToAll), then
            # the last quarter's projection + its LayerNorm
            ln_unit(2)
            op_unit(3, 0)
            op_unit(3, 1)
            ln_unit(3)

    nc.compile()
    return nc


_NC_CACHE = {}


def _get_nc(S=2048, B=2, D=1024):
    key = (S, B, D)
    if key not in _NC_CACHE:
        _NC_CACHE[key] = build_bert_kernel(S, B, D)
    return _NC_CACHE[key]


def make_in_maps(query_tensor, key_tensor, value_tensor, Wq, bq, Wk, bk,
                 Wv, bv, Wo, bo, ln_w, ln_b):
    S, B, D = query_tensor.shape
    NTOK = S * B
    DL = (H // NCORES) * DH

    def bm(x):  # (S, B, D) -> batch-major (B*S, D) float32
        return np.ascontiguousarray(
            np.asarray(x, np.float32).transpose(1, 0, 2).reshape(NTOK, D))

    def bmT_tiled(x):  # (S, B, D) -> [CCH, B, 128, S] tile-contiguous
        a = bm(x).T.astype(BF16_NP)                      # (D, B*S)
        a = a.reshape(D // 128, 128, B, S).transpose(0, 2, 1, 3)
        return np.ascontiguousarray(a)

    xq = bm(query_tensor)
    xqT = bmT_tiled(query_tensor)
    xkT = bmT_tiled(key_tensor)
    xvT = bmT_tiled(value_tensor)
    woT = np.ascontiguousarray(
        np.asarray(Wo, np.float32).T.astype(BF16_NP)).reshape(D // 128, 128, D)
    f32 = lambda a: np.ascontiguousarray(np.asarray(a, np.float32))
    bf16T = lambda a: np.ascontiguousarray(
        np.asarray(a, np.float32).T.astype(BF16_NP))
    in_maps = []
    for c in range(NCORES):
        sl = slice(c * DL, (c + 1) * DL)
        rs = np.concatenate([xq[q * 1024 + c * 128:q * 1024 + (c + 1) * 128]
                             for q in range(NTOK // 1024)], axis=0)
        in_maps.append({
            "xqT": xqT, "xkT": xkT, "xvT": xvT,
            "wqT": bf16T(Wq[sl]), "wkT": bf16T(Wk[sl]),
            "wvT": bf16T(Wv[sl]), "woT": woT,
            "bq": f32(bq[sl]).reshape(DL, 1),
            "bk": f32(bk[sl]).reshape(DL, 1),
            "bv": f32(bv[sl]).reshape(1, DL),
            "bo": f32(bo).reshape(1, D),
            "lnw": f32(ln_w).reshape(1, D),
            "lnb": f32(ln_b).reshape(1, D),
            "resid": np.ascontiguousarray(rs),
        })
    return in_maps


def assemble_output(results, S, B, D):
    NTOK = S * B
    full = np.empty((NTOK, D), np.float32)
    for c, r in enumerate(results):
        o = r["out"]  # (512, D): 4 x 128-token quarter slices
        for q in range(NTOK // 1024):
            full[q * 1024 + c * 128:q * 1024 + (c + 1) * 128] = \
                o[q * 128:(q + 1) * 128]
    return np.ascontiguousarray(
        full.reshape(B, S, D).transpose(1, 0, 2))


def kernel(**inputs):
    S, B, D = inputs["query_tensor"].shape
    nc = _get_nc(S, B, D)
    in_maps = make_in_maps(**inputs)
    res = run_bass_kernel_spmd(nc, in_maps, list(range(NCORES)))
    return assemble_output(res.results, S, B, D)


# revision 47
# speedup vs baseline: 1.1510x; 1.1178x over previous
"""BertAttention Trainium2 kernel — 8-core SPMD, v2.

Sharding: each core owns 2 heads (128 of the 1024 feature dims) and a
512-token output slice (4 x 128-token quarters, one per 1024-token span).

Key design points (vs v1):
  - Global slot schedule: one 128-key score chunk per slot; ctx matmuls
    lag scores by 2 slots (h0) / 3 slots (h1); exp (ACT engine) is one
    slot behind scores with double-buffered PSUM so the PE never stalls
    on the activation engine and keeps its fast p-state.
  - QKV projections for batch 1 and the output projection run as filler
    matmul units inside attention slots that would otherwise idle.
  - Four quarter AllToAlls (256KB each) replace the AllGather (7MB/core):
    each core receives exactly the full-width ctx^T for its own tokens,
    so phase 3 is rank-static and overlaps with attention.

Host passes activations/weights pre-transposed and pre-cast to bf16
(feature-major), so the device spends no time on casts/transposes.
"""

import os
import sys

for _p in ("/opt/trn_rl_repo", "/root/.axon_site/_ro/trn_rl_repo"):
    if os.path.isdir(_p) and _p not in sys.path:
        sys.path.append(_p)

import ml_dtypes
import numpy as np

# Shim antenv.axon_hooks (absent in some images): bass_utils imports it
# unconditionally when tracing is requested via env.
try:
    import antenv.axon_hooks  # noqa: F401
except Exception:
    import types as _types
    try:
        import antenv as _antenv
        _m = _types.ModuleType("antenv.axon_hooks")
        _m._hook = None
        _m.set_axon_ntff_profile_hook = lambda h, _m=_m: setattr(_m, "_hook", h)
        _m.get_axon_ntff_profile_hook = lambda _m=_m: _m._hook
        sys.modules["antenv.axon_hooks"] = _m
        _antenv.axon_hooks = _m
    except Exception:
        pass

import concourse.bass as bass  # noqa: F401
import concourse.tile as tile
from concourse import bacc, mybir
from concourse.bass_utils import run_bass_kernel_spmd

F32 = mybir.dt.float32
BF16 = mybir.dt.bfloat16
BF16_NP = ml_dtypes.bfloat16

NCORES = 8
H = 16   # heads total
DH = 64  # head dim
LN_EPS = 1e-12


def build_bert_kernel(S=2048, B=2, D=1024):
    P = 128
    NTOK = S * B              # 4096 batch-major tokens
    TPC = NTOK // NCORES      # 512 output tokens per core (4 quarters)
    CCH = D // P              # 8 contraction chunks
    HPC = H // NCORES         # 2 heads per core
    DL = HPC * DH             # 128 local feature dims
    NJ = S // P               # 16 key chunks per batch
    NI = S // 512             # 4 query blocks per batch
    NSLOT = B * NI * NJ       # 128 score chunk-slots
    NQTR = NTOK // 1024       # 4 quarters
    NVT = NTOK // P           # 32 v token tiles

    nc = bacc.Bacc("TRN2", target_bir_lowering=False, debug=False,
                   num_devices=NCORES)

    def din(name, shape, dt=F32):
        return nc.dram_tensor(name, list(shape), dt, kind="ExternalInput").ap()

    # x inputs come host-tiled: [CCH, B, 128, S] so each (chunk, batch)
    # tile is one fully-contiguous 512KB DMA
    xqT = din("xqT", (CCH, B, P, S), BF16)
    xkT = din("xkT", (CCH, B, P, S), BF16)
    xvT = din("xvT", (CCH, B, P, S), BF16)
    wqT = din("wqT", (D, DL), BF16)
    wkT = din("wkT", (D, DL), BF16)
    wvT = din("wvT", (D, DL), BF16)
    woT = din("woT", (CCH, P, D), BF16)
    bq = din("bq", (DL, 1))
    bk = din("bk", (DL, 1))
    bv = din("bv", (1, DL))
    bo = din("bo", (1, D))
    lnw = din("lnw", (1, D))
    lnb = din("lnb", (1, D))
    resid = din("resid", (TPC, D))
    out = nc.dram_tensor("out", [TPC, D], F32, kind="ExternalOutput").ap()

    # per-quarter exchange buffers: piece p of a2a[q] = ctx^T of this
    # core's heads for tokens [q*1024 + p*128, +128) -> destined core p.
    a2a = [nc.dram_tensor(f"a2a{q}", [NCORES, P, P], BF16).ap()
           for q in range(NQTR)]
    ag = [nc.dram_tensor(f"ag{q}", [NCORES, P, P], BF16).ap()
          for q in range(NQTR)]
    GRP = [list(range(NCORES))]

    with tile.TileContext(nc) as tc:
        with (
            tc.tile_pool(name="persist", bufs=1) as persist,
            tc.tile_pool(name="small", bufs=1) as small,
            tc.tile_pool(name="xT", bufs=1) as xt_pool,
            tc.tile_pool(name="work", bufs=1) as work,
            tc.tile_pool(name="ps_sc", bufs=1, space="PSUM") as ps_sc,
            tc.tile_pool(name="ps_cps", bufs=1, space="PSUM") as ps_cps,
        ):
            SCB = 3  # scores/filler PSUM ring depth (3 x 2 banks)

            def sc_tile():
                return ps_sc.tile([P, 2 * 512], F32, name="sc",
                                  tag="sc", bufs=SCB)
            # ---- weights into SBUF (woT deferred into the slot loop) ----
            wqT_sb = persist.tile([P, CCH, DL], BF16)
            wkT_sb = persist.tile([P, CCH, DL], BF16)
            wvT_sb = persist.tile([P, CCH, DL], BF16)
            for wi, (w_d, w_sb) in enumerate(
                    ((wkT, wkT_sb), (wqT, wqT_sb), (wvT, wvT_sb))):
                for c in range(CCH):
                    (nc.sync if (wi + c) % 2 else nc.gpsimd).dma_start(
                        out=w_sb[:, c, :], in_=w_d[c * P:(c + 1) * P, :])
            woT_sb = persist.tile([P, CCH, D], BF16)

            def load_woT():
                for c in range(CCH):
                    nc.sync.dma_start(out=woT_sb[:, c, :], in_=woT[c, :, :])

            # ---- constant / bias tiles ----
            bq_sb = small.tile([P, 1], F32)
            bk_sb = small.tile([P, 1], F32)
            nc.sync.dma_start(out=bq_sb, in_=bq)
            nc.sync.dma_start(out=bk_sb, in_=bk)
            bv_bc = small.tile([P, DL], F32)
            nc.gpsimd.dma_start(out=bv_bc, in_=bv.to_broadcast((P, DL)))
            bo_bc = small.tile([P, D], F32)
            nc.gpsimd.dma_start(out=bo_bc, in_=bo.to_broadcast((P, D)))
            lnw_bc = small.tile([P, D], F32)
            nc.gpsimd.dma_start(out=lnw_bc, in_=lnw.to_broadcast((P, D)))
            lnb_bc = small.tile([P, D], F32)
            nc.gpsimd.dma_start(out=lnb_bc, in_=lnb.to_broadcast((P, D)))
            eps_sb = small.tile([P, 1], F32)
            nc.vector.memset(eps_sb, LN_EPS)

            # ---- persistent activation buffers ----
            qT_sb = persist.tile([P, NTOK], BF16)   # [dloc, tok]
            kT_sb = persist.tile([P, NTOK], BF16)
            v_sb = persist.tile([P, NVT, HPC * (DH + 1)], BF16)
            nc.vector.memset(v_sb[:, :, DH:DH + 1], 1.0)
            nc.vector.memset(v_sb[:, :, 2 * DH + 1:2 * DH + 2], 1.0)
            ERD = 8
            e_ring = persist.tile([P, ERD, 2 * 512], BF16)  # exp ring
            ctxF = [persist.tile([P, CCH, P], BF16, name=f"ctxF{q}")
                    for q in range(NQTR)]
            y_sb = [persist.tile([P, D], F32, name=f"y{q}")
                    for q in range(NQTR)]

            # ---- x tiles: one contiguous 512KB DMA per (proj, chunk,
            # batch) in prologue consumption order (k, q, v; b0 then b1).
            # Ring-gated b1 issues resolve against prologue-consumed b0
            # buffers, so no sequencer ever blocks against in-loop work.
            xt = {}
            ei = 0
            dmaq = (nc.sync, nc.scalar, nc.gpsimd)
            xd = {0: xqT, 1: xkT, 2: xvT}
            for b in range(B):
                for ti in (1, 0, 2):        # k, q, v
                    for c in range(CCH):
                        t = xt_pool.tile([P, S], BF16,
                                         name=f"xT{ti}_{c}_{b}",
                                         tag="xT", bufs=16)
                        xt[(ti, c, b)] = t
                        dmaq[ei % 3].dma_start(out=t, in_=xd[ti][c, b, :, :])
                        ei += 1

            # ================= unit builders =================
            def qk_unit(ti, b, n):
                # projection of 512 tokens onto this core's 128 q/k dims
                w_sb, b_sb, o_sb = ((wqT_sb, bq_sb, qT_sb),
                                    (wkT_sb, bk_sb, kT_sb))[ti]
                ps = sc_tile()[:, 0:512]
                for c in range(CCH):
                    nc.tensor.matmul(ps, w_sb[:, c, :],
                                     xt[(ti, c, b)][:, n * 512:(n + 1) * 512],
                                     start=(c == 0), stop=(c == CCH - 1))
                tok0 = b * S + n * 512
                nc.vector.tensor_scalar_add(o_sb[:, tok0:tok0 + 512], ps, b_sb)

            def v_unit(b, nt):
                # v projection for 4 token tiles (512 tokens)
                ps = sc_tile()[:, 0:512]
                for k in range(4):
                    itl = nt * 4 + k
                    for c in range(CCH):
                        nc.tensor.matmul(
                            ps[:, k * P:(k + 1) * P],
                            xt[(2, c, b)][:, itl * P:(itl + 1) * P],
                            wvT_sb[:, c, :],
                            start=(c == 0), stop=(c == CCH - 1))
                for k in range(4):
                    it = b * (S // P) + nt * 4 + k
                    dst = v_sb[:, it, :].rearrange(
                        "p (h x) -> p h x", h=HPC)[:, :, 0:DH]
                    src = ps[:, k * P:(k + 1) * P].rearrange(
                        "p (h x) -> p h x", h=HPC)
                    bvr = bv_bc.rearrange("p (h x) -> p h x", h=HPC)
                    nc.vector.tensor_add(dst, src, bvr)

            resid_sb = {}

            def op_unit(i2, n):
                # output projection: 128 tokens x 512 output dims
                cF = ctxF[i2]
                ps = sc_tile()[:, 0:512]
                for c in range(CCH):
                    nc.tensor.matmul(ps, cF[:, c, :],
                                     woT_sb[:, c, n * 512:(n + 1) * 512],
                                     start=(c == 0), stop=(c == CCH - 1))
                if n == 0:
                    rt = work.tile([P, D], F32, tag="resid", bufs=2,
                                   name=f"resid{i2}")
                    resid_sb[i2] = rt
                    nc.sync.dma_start(out=rt,
                                      in_=resid[i2 * P:(i2 + 1) * P, :])
                sl = slice(n * 512, (n + 1) * 512)
                y = y_sb[i2]
                nc.vector.tensor_add(y[:, sl], ps, bo_bc[:, sl])
                nc.vector.tensor_add(y[:, sl], y[:, sl], resid_sb[i2][:, sl])

            def ln_unit(i2):
                y = y_sb[i2]
                y3 = y.rearrange("p (g d) -> p g d", g=2)
                stats = work.tile([P, 2, 6], F32, tag="stats", bufs=2)
                for g in range(2):
                    nc.vector.bn_stats(out=stats[:, g, :], in_=y3[:, g, :])
                mv = work.tile([P, 2], F32, tag="mv", bufs=2)
                nc.vector.bn_aggr(out=mv, in_=stats)
                std = work.tile([P, 1], F32, tag="std", bufs=2)
                nc.scalar.activation(std, mv[:, 1:2],
                                     mybir.ActivationFunctionType.Sqrt,
                                     bias=eps_sb)
                rstd = work.tile([P, 1], F32, tag="rstd", bufs=2)
                nc.vector.reciprocal(rstd, std)
                t32 = work.tile([P, D], F32, tag="t32", bufs=1)
                nc.vector.tensor_scalar(
                    out=t32, in0=y, scalar1=mv[:, 0:1], scalar2=rstd,
                    op0=mybir.AluOpType.subtract, op1=mybir.AluOpType.mult)
                of = work.tile([P, D], F32, tag="of", bufs=2)
                nc.vector.tensor_mul(of, t32, lnw_bc)
                nc.vector.tensor_add(of, of, lnb_bc)
                nc.sync.dma_start(out=out[i2 * P:(i2 + 1) * P, :], in_=of)

            # ---- filler placement: slot -> list of unit closures ----
            # units are placed just after their input DMAs can have
            # landed and just before their outputs are first consumed.
            fillers = {}

            def add_filler(s, fn):
                fillers.setdefault(s, []).append(fn)

            # b1 fillers: placed after their x DMAs can have landed
            # (b1 loads flow once the prologue frees the xt ring)
            sched = [(20, load_woT),
                     (26, lambda: qk_unit(1, 1, 0)),
                     (28, lambda: qk_unit(1, 1, 1)),
                     (30, lambda: qk_unit(1, 1, 2)),
                     (32, lambda: qk_unit(1, 1, 3)),
                     (34, lambda: qk_unit(0, 1, 0)),
                     (36, lambda: qk_unit(0, 1, 1)),
                     (38, lambda: qk_unit(0, 1, 2)),
                     (40, lambda: qk_unit(0, 1, 3)),
                     (44, lambda: v_unit(1, 0)),
                     (46, lambda: v_unit(1, 1)),
                     (48, lambda: v_unit(1, 2)),
                     (50, lambda: v_unit(1, 3))]
            for s, fn in sched:
                add_filler(s, fn)
            # out-projection units go as late as their deadlines allow:
            # each AllToAll then has ~60us of slack to absorb cross-core
            # skew before the PE stream depends on its result
            add_filler(80, lambda: op_unit(0, 0))
            add_filler(84, lambda: op_unit(0, 1))
            add_filler(88, lambda: op_unit(1, 0))
            add_filler(92, lambda: op_unit(1, 1))
            add_filler(108, lambda: op_unit(2, 0))
            add_filler(112, lambda: op_unit(2, 1))
            add_filler(118, lambda: ln_unit(0))
            add_filler(118, lambda: ln_unit(1))

            # ================= main slot loop =================
            cps = [None, None]        # live ctx PSUM tile per head
            ctxo = {}                 # (b, i) -> staging tile

            def scores_step(s):
                b, r = divmod(s, NI * NJ)
                i, j = divmod(r, NJ)
                sc = sc_tile()
                jc0 = b * S + j * P
                ic0 = b * S + i * 512
                for h in range(HPC):
                    nc.tensor.matmul(
                        sc[:, h * 512:(h + 1) * 512],
                        kT_sb[h * DH:(h + 1) * DH, jc0:jc0 + P],
                        qT_sb[h * DH:(h + 1) * DH, ic0:ic0 + 512])
                nc.scalar.activation(e_ring[:, s % ERD, :], sc,
                                     mybir.ActivationFunctionType.Exp)

            def ctx_step(c, h):
                b, r = divmod(c, NI * NJ)
                i, j = divmod(r, NJ)
                vt = b * NJ + j
                if j == 0:
                    cps[h] = ps_cps.tile([DH + 1, 512], F32, name=f"cps{h}",
                                         tag=f"cps{h}", bufs=1)
                nc.tensor.matmul(
                    cps[h],
                    v_sb[:, vt, h * (DH + 1):(h + 1) * (DH + 1)],
                    e_ring[:, c % ERD, h * 512:(h + 1) * 512],
                    start=(j == 0), stop=(j == NJ - 1))
                if j != NJ - 1:
                    return
                # ---- softmax normalize + stage for exchange ----
                # single fast PSUM->SBUF copy frees the cps bank; the
                # normalize chain then runs off the PE critical path
                if h == 0:
                    ctxo[(b, i)] = work.tile([P, 512], BF16, tag="ctxo",
                                             bufs=2, name=f"ctxo{b}_{i}")
                co = ctxo[(b, i)]
                cs = work.tile([DH, 512], F32, tag="cs", bufs=2)
                nc.vector.tensor_copy(cs, cps[h][0:DH, :])
                ssum = work.tile([1, 512], F32, tag="ssum", bufs=2)
                nc.vector.tensor_copy(ssum, cps[h][DH:DH + 1, :])
                rcp = work.tile([1, 512], F32, tag="rcp", bufs=2)
                nc.vector.reciprocal_approx_fast(rcp, ssum)
                rbc = work.tile([DH, 512], F32, tag="rbc", bufs=2)
                nc.gpsimd.partition_broadcast(rbc, rcp)
                nc.vector.tensor_mul(co[h * DH:(h + 1) * DH, :],
                                     cs, rbc)
                if h == HPC - 1:
                    tok0 = b * S + i * 512
                    q = tok0 // 1024
                    p0 = (tok0 % 1024) // P
                    for t4 in range(4):
                        nc.gpsimd.dma_start(
                            out=a2a[q][p0 + t4, :, :],
                            in_=co[:, t4 * P:(t4 + 1) * P])
                    if i % 2 == 1:  # quarter complete -> exchange
                        nc.gpsimd.collective_compute(
                            "AllToAll", mybir.AluOpType.bypass,
                            replica_groups=GRP,
                            ins=[a2a[q].opt()], outs=[ag[q].opt()])
                        for cc in range(CCH):
                            nc.sync.dma_start(out=ctxF[q][:, cc, :],
                                              in_=ag[q][cc, :, :])

            # prologue: full QKV(b0) (solid PE block, no ring hazards)
            for n in range(4):
                qk_unit(1, 0, n)
            for n in range(4):
                qk_unit(0, 0, n)
            for nt in range(4):
                v_unit(0, nt)

            for s in range(NSLOT + 3):
                if s < NSLOT:
                    scores_step(s)
                if 0 <= s - 2 < NSLOT:
                    ctx_step(s - 2, 0)
                if 0 <= s - 3 < NSLOT:
                    ctx_step(s - 3, 1)
                for fn in fillers.get(s, ()):
                    fn()

            # epilogue: LN(2) first (overlaps the last AllToAll), then
            # the last quarter's projection + its LayerNorm
            ln_unit(2)
            op_unit(3, 0)
            op_unit(3, 1)
            ln_unit(3)

    nc.compile()
    return nc


_NC_CACHE = {}


def _get_nc(S=2048, B=2, D=1024):
    key = (S, B, D)
    if key not in _NC_CACHE:
        _NC_CACHE[key] = build_bert_kernel(S, B, D)
    return _NC_CACHE[key]


def make_in_maps(query_tensor, key_tensor, value_tensor, Wq, bq, Wk, bk,
                 Wv, bv, Wo, bo, ln_w, ln_b):
    S, B, D = query_tensor.shape
    NTOK = S * B
    DL = (H // NCORES) * DH

    def bm(x):  # (S, B, D) -> batch-major (B*S, D) float32
        return np.ascontiguousarray(
            np.asarray(x, np.float32).transpose(1, 0, 2).reshape(NTOK, D))

    def bmT_tiled(x):  # (S, B, D) -> [CCH, B, 128, S] tile-contiguous
        a = bm(x).T.astype(BF16_NP)                      # (D, B*S)
        a = a.reshape(D // 128, 128, B, S).transpose(0, 2, 1, 3)
        return np.ascontiguousarray(a)

    xq = bm(query_tensor)
    xqT = bmT_tiled(query_tensor)
    xkT = bmT_tiled(key_tensor)
    xvT = bmT_tiled(value_tensor)
    woT = np.ascontiguousarray(
        np.asarray(Wo, np.float32).T.astype(BF16_NP)).reshape(D // 128, 128, D)
    f32 = lambda a: np.ascontiguousarray(np.asarray(a, np.float32))
    bf16T = lambda a: np.ascontiguousarray(
        np.asarray(a, np.float32).T.astype(BF16_NP))
    in_maps = []
    for c in range(NCORES):
        sl = slice(c * DL, (c + 1) * DL)
        rs = np.concatenate([xq[q * 1024 + c * 128:q * 1024 + (c + 1) * 128]
                             for q in range(NTOK // 1024)], axis=0)
        in_maps.append({
            "xqT": xqT, "xkT": xkT, "xvT": xvT,
            "wqT": bf16T(Wq[sl]), "wkT": bf16T(Wk[sl]),
            "wvT": bf16T(Wv[sl]), "woT": woT,
            "bq": f32(bq[sl]).reshape(DL, 1),
            "bk": f32(bk[sl]).reshape(DL, 1),
            "bv": f32(bv[sl]).reshape(1, DL),
            "bo": f32(bo).reshape(1, D),
            "lnw": f32(ln_w).reshape(1, D),
            "lnb": f32(ln_b).reshape(1, D),
            "resid": np.ascontiguousarray(rs),
        })
    return in_maps


def assemble_output(results, S, B, D):
    NTOK = S * B
    full = np.empty((NTOK, D), np.float32)
    for c, r in enumerate(results):
        o = r["out"]  # (512, D): 4 x 128-token quarter slices
        for q in range(NTOK // 1024):
            full[q * 1024 + c * 128:q * 1024 + (c + 1) * 128] = \
                o[q * 128:(q + 1) * 128]
    return np.ascontiguousarray(
        full.reshape(B, S, D).transpose(1, 0, 2))


def kernel(**inputs):
    S, B, D = inputs["query_tensor"].shape
    nc = _get_nc(S, B, D)
    in_maps = make_in_maps(**inputs)
    res = run_bass_kernel_spmd(nc, in_maps, list(range(NCORES)))
    return assemble_output(res.results, S, B, D)


# revision 48
# speedup vs baseline: 1.2318x; 1.0701x over previous
"""BertAttention Trainium2 kernel — 8-core SPMD, v2.

Sharding: each core owns 2 heads (128 of the 1024 feature dims) and a
512-token output slice (4 x 128-token quarters, one per 1024-token span).

Key design points (vs v1):
  - Global slot schedule: one 128-key score chunk per slot; ctx matmuls
    lag scores by 2 slots (h0) / 3 slots (h1); exp (ACT engine) is one
    slot behind scores with double-buffered PSUM so the PE never stalls
    on the activation engine and keeps its fast p-state.
  - QKV projections for batch 1 and the output projection run as filler
    matmul units inside attention slots that would otherwise idle.
  - Four quarter AllToAlls (256KB each) replace the AllGather (7MB/core):
    each core receives exactly the full-width ctx^T for its own tokens,
    so phase 3 is rank-static and overlaps with attention.

Host passes activations/weights pre-transposed and pre-cast to bf16
(feature-major), so the device spends no time on casts/transposes.
"""

import os
import sys

for _p in ("/opt/trn_rl_repo", "/root/.axon_site/_ro/trn_rl_repo"):
    if os.path.isdir(_p) and _p not in sys.path:
        sys.path.append(_p)

import ml_dtypes
import numpy as np

# Shim antenv.axon_hooks (absent in some images): bass_utils imports it
# unconditionally when tracing is requested via env.
try:
    import antenv.axon_hooks  # noqa: F401
except Exception:
    import types as _types
    try:
        import antenv as _antenv
        _m = _types.ModuleType("antenv.axon_hooks")
        _m._hook = None
        _m.set_axon_ntff_profile_hook = lambda h, _m=_m: setattr(_m, "_hook", h)
        _m.get_axon_ntff_profile_hook = lambda _m=_m: _m._hook
        sys.modules["antenv.axon_hooks"] = _m
        _antenv.axon_hooks = _m
    except Exception:
        pass

import concourse.bass as bass  # noqa: F401
import concourse.tile as tile
from concourse import bacc, mybir
from concourse.bass_utils import run_bass_kernel_spmd

F32 = mybir.dt.float32
BF16 = mybir.dt.bfloat16
BF16_NP = ml_dtypes.bfloat16

NCORES = 8
H = 16   # heads total
DH = 64  # head dim
LN_EPS = 1e-12


def build_bert_kernel(S=2048, B=2, D=1024):
    P = 128
    NTOK = S * B              # 4096 batch-major tokens
    TPC = NTOK // NCORES      # 512 output tokens per core (4 quarters)
    CCH = D // P              # 8 contraction chunks
    HPC = H // NCORES         # 2 heads per core
    DL = HPC * DH             # 128 local feature dims
    NJ = S // P               # 16 key chunks per batch
    NI = S // 512             # 4 query blocks per batch
    NSLOT = B * NI * NJ       # 128 score chunk-slots
    NQTR = NTOK // 1024       # 4 quarters
    NVT = NTOK // P           # 32 v token tiles

    nc = bacc.Bacc("TRN2", target_bir_lowering=False, debug=False,
                   num_devices=NCORES)

    def din(name, shape, dt=F32):
        return nc.dram_tensor(name, list(shape), dt, kind="ExternalInput").ap()

    # x inputs come host-tiled: [CCH, B, 128, S] so each (chunk, batch)
    # tile is one fully-contiguous 512KB DMA
    xqT = din("xqT", (CCH, B, P, S), BF16)
    xkT = din("xkT", (CCH, B, P, S), BF16)
    xvT = din("xvT", (CCH, B, P, S), BF16)
    wqT = din("wqT", (D, DL), BF16)
    wkT = din("wkT", (D, DL), BF16)
    wvT = din("wvT", (D, DL), BF16)
    woT = din("woT", (CCH, P, D), BF16)
    bq = din("bq", (DL, 1))
    bk = din("bk", (DL, 1))
    bv = din("bv", (1, DL))
    bo = din("bo", (1, D))
    lnw = din("lnw", (1, D))
    lnb = din("lnb", (1, D))
    resid = din("resid", (TPC, D))
    out = nc.dram_tensor("out", [TPC, D], F32, kind="ExternalOutput").ap()

    # per-quarter exchange buffers: piece p of a2a[q] = ctx^T of this
    # core's heads for tokens [q*1024 + p*128, +128) -> destined core p.
    a2a = [nc.dram_tensor(f"a2a{q}", [NCORES, P, P], BF16).ap()
           for q in range(NQTR)]
    ag = [nc.dram_tensor(f"ag{q}", [NCORES, P, P], BF16).ap()
          for q in range(NQTR)]
    GRP = [list(range(NCORES))]

    with tile.TileContext(nc) as tc:
        with (
            tc.tile_pool(name="persist", bufs=1) as persist,
            tc.tile_pool(name="small", bufs=1) as small,
            tc.tile_pool(name="xT", bufs=1) as xt_pool,
            tc.tile_pool(name="work", bufs=1) as work,
            tc.tile_pool(name="ps_sc", bufs=1, space="PSUM") as ps_sc,
            tc.tile_pool(name="ps_cps", bufs=1, space="PSUM") as ps_cps,
        ):
            SCB = 3  # scores/filler PSUM ring depth (3 x 2 banks)

            def sc_tile():
                return ps_sc.tile([P, 2 * 512], F32, name="sc",
                                  tag="sc", bufs=SCB)
            # ---- weights into SBUF (woT deferred into the slot loop) ----
            wqT_sb = persist.tile([P, CCH, DL], BF16)
            wkT_sb = persist.tile([P, CCH, DL], BF16)
            wvT_sb = persist.tile([P, CCH, DL], BF16)
            for wi, (w_d, w_sb) in enumerate(
                    ((wkT, wkT_sb), (wqT, wqT_sb), (wvT, wvT_sb))):
                for c in range(CCH):
                    (nc.sync if (wi + c) % 2 else nc.gpsimd).dma_start(
                        out=w_sb[:, c, :], in_=w_d[c * P:(c + 1) * P, :])
            woT_sb = persist.tile([P, CCH, D], BF16)

            def load_woT():
                for c in range(CCH):
                    nc.sync.dma_start(out=woT_sb[:, c, :], in_=woT[c, :, :])

            # ---- constant / bias tiles ----
            bq_sb = small.tile([P, 1], F32)
            bk_sb = small.tile([P, 1], F32)
            nc.sync.dma_start(out=bq_sb, in_=bq)
            nc.sync.dma_start(out=bk_sb, in_=bk)
            bv_bc = small.tile([P, DL], F32)
            nc.gpsimd.dma_start(out=bv_bc, in_=bv.to_broadcast((P, DL)))
            bo_bc = small.tile([P, D], F32)
            nc.gpsimd.dma_start(out=bo_bc, in_=bo.to_broadcast((P, D)))
            lnw_bc = small.tile([P, D], F32)
            nc.gpsimd.dma_start(out=lnw_bc, in_=lnw.to_broadcast((P, D)))
            lnb_bc = small.tile([P, D], F32)
            nc.gpsimd.dma_start(out=lnb_bc, in_=lnb.to_broadcast((P, D)))
            eps_sb = small.tile([P, 1], F32)
            nc.vector.memset(eps_sb, LN_EPS)

            # ---- persistent activation buffers ----
            qT_sb = persist.tile([P, NTOK], BF16)   # [dloc, tok]
            kT_sb = persist.tile([P, NTOK], BF16)
            v_sb = persist.tile([P, NVT, HPC * (DH + 1)], BF16)
            nc.vector.memset(v_sb[:, :, DH:DH + 1], 1.0)
            nc.vector.memset(v_sb[:, :, 2 * DH + 1:2 * DH + 2], 1.0)
            ERD = 8
            e_ring = persist.tile([P, ERD, 2 * 512], BF16)  # exp ring
            ctxF = [persist.tile([P, CCH, P], BF16, name=f"ctxF{q}")
                    for q in range(NQTR)]
            y_sb = [persist.tile([P, D], F32, name=f"y{q}")
                    for q in range(NQTR)]

            # ---- x tiles: one contiguous 512KB DMA per (proj, chunk,
            # batch) in prologue consumption order (k, q, v; b0 then b1).
            # Ring-gated b1 issues resolve against prologue-consumed b0
            # buffers, so no sequencer ever blocks against in-loop work.
            xt = {}
            ei = 0
            dmaq = (nc.sync, nc.scalar, nc.gpsimd)
            xd = {0: xqT, 1: xkT, 2: xvT}
            for b in range(B):
                for ti in (1, 0, 2):        # k, q, v
                    for c in range(CCH):
                        t = xt_pool.tile([P, S], BF16,
                                         name=f"xT{ti}_{c}_{b}",
                                         tag="xT", bufs=16)
                        xt[(ti, c, b)] = t
                        dmaq[ei % 3].dma_start(out=t, in_=xd[ti][c, b, :, :])
                        ei += 1

            # ================= unit builders =================
            def qk_unit(ti, b, n):
                # projection of 512 tokens onto this core's 128 q/k dims
                w_sb, b_sb, o_sb = ((wqT_sb, bq_sb, qT_sb),
                                    (wkT_sb, bk_sb, kT_sb))[ti]
                ps = sc_tile()[:, 0:512]
                for c in range(CCH):
                    nc.tensor.matmul(ps, w_sb[:, c, :],
                                     xt[(ti, c, b)][:, n * 512:(n + 1) * 512],
                                     start=(c == 0), stop=(c == CCH - 1))
                tok0 = b * S + n * 512
                nc.vector.tensor_scalar_add(o_sb[:, tok0:tok0 + 512], ps, b_sb)

            def v_unit(b, nt):
                # v projection for 4 token tiles (512 tokens)
                ps = sc_tile()[:, 0:512]
                for k in range(4):
                    itl = nt * 4 + k
                    for c in range(CCH):
                        nc.tensor.matmul(
                            ps[:, k * P:(k + 1) * P],
                            xt[(2, c, b)][:, itl * P:(itl + 1) * P],
                            wvT_sb[:, c, :],
                            start=(c == 0), stop=(c == CCH - 1))
                for k in range(4):
                    it = b * (S // P) + nt * 4 + k
                    dst = v_sb[:, it, :].rearrange(
                        "p (h x) -> p h x", h=HPC)[:, :, 0:DH]
                    src = ps[:, k * P:(k + 1) * P].rearrange(
                        "p (h x) -> p h x", h=HPC)
                    bvr = bv_bc.rearrange("p (h x) -> p h x", h=HPC)
                    nc.vector.tensor_add(dst, src, bvr)

            resid_sb = {}

            def op_unit(i2, n):
                # output projection: 128 tokens x 512 output dims
                cF = ctxF[i2]
                ps = sc_tile()[:, 0:512]
                for c in range(CCH):
                    nc.tensor.matmul(ps, cF[:, c, :],
                                     woT_sb[:, c, n * 512:(n + 1) * 512],
                                     start=(c == 0), stop=(c == CCH - 1))
                if n == 0:
                    rt = work.tile([P, D], F32, tag="resid", bufs=2,
                                   name=f"resid{i2}")
                    resid_sb[i2] = rt
                    nc.sync.dma_start(out=rt,
                                      in_=resid[i2 * P:(i2 + 1) * P, :])
                sl = slice(n * 512, (n + 1) * 512)
                y = y_sb[i2]
                nc.vector.tensor_add(y[:, sl], ps, bo_bc[:, sl])
                nc.vector.tensor_add(y[:, sl], y[:, sl], resid_sb[i2][:, sl])

            def ln_unit(i2):
                y = y_sb[i2]
                y3 = y.rearrange("p (g d) -> p g d", g=2)
                stats = work.tile([P, 2, 6], F32, tag="stats", bufs=2)
                for g in range(2):
                    nc.vector.bn_stats(out=stats[:, g, :], in_=y3[:, g, :])
                mv = work.tile([P, 2], F32, tag="mv", bufs=2)
                nc.vector.bn_aggr(out=mv, in_=stats)
                std = work.tile([P, 1], F32, tag="std", bufs=2)
                nc.scalar.activation(std, mv[:, 1:2],
                                     mybir.ActivationFunctionType.Sqrt,
                                     bias=eps_sb)
                rstd = work.tile([P, 1], F32, tag="rstd", bufs=2)
                nc.vector.reciprocal(rstd, std)
                t32 = work.tile([P, D], F32, tag="t32", bufs=1)
                nc.vector.tensor_scalar(
                    out=t32, in0=y, scalar1=mv[:, 0:1], scalar2=rstd,
                    op0=mybir.AluOpType.subtract, op1=mybir.AluOpType.mult)
                of = work.tile([P, D], F32, tag="of", bufs=2)
                nc.vector.tensor_mul(of, t32, lnw_bc)
                nc.vector.tensor_add(of, of, lnb_bc)
                nc.sync.dma_start(out=out[i2 * P:(i2 + 1) * P, :], in_=of)

            # ---- filler placement: slot -> list of unit closures ----
            # units are placed just after their input DMAs can have
            # landed and just before their outputs are first consumed.
            fillers = {}

            def add_filler(s, fn):
                fillers.setdefault(s, []).append(fn)

            # b1 fillers: placed after their x DMAs can have landed
            # (b1 loads flow once the prologue frees the xt ring)
            sched = [(20, load_woT),
                     (26, lambda: qk_unit(1, 1, 0)),
                     (28, lambda: qk_unit(1, 1, 1)),
                     (30, lambda: qk_unit(1, 1, 2)),
                     (32, lambda: qk_unit(1, 1, 3)),
                     (34, lambda: qk_unit(0, 1, 0)),
                     (36, lambda: qk_unit(0, 1, 1)),
                     (38, lambda: qk_unit(0, 1, 2)),
                     (40, lambda: qk_unit(0, 1, 3)),
                     (44, lambda: v_unit(1, 0)),
                     (46, lambda: v_unit(1, 1)),
                     (48, lambda: v_unit(1, 2)),
                     (50, lambda: v_unit(1, 3))]
            for s, fn in sched:
                add_filler(s, fn)
            # out-projection units go as late as their deadlines allow:
            # each AllToAll then has maximal slack to absorb cross-core
            # skew (20-45us/run) before the PE stream depends on it
            add_filler(96, lambda: op_unit(0, 0))
            add_filler(100, lambda: op_unit(0, 1))
            add_filler(104, lambda: op_unit(1, 0))
            add_filler(108, lambda: op_unit(1, 1))
            add_filler(118, lambda: op_unit(2, 0))
            add_filler(122, lambda: op_unit(2, 1))
            add_filler(126, lambda: ln_unit(0))
            add_filler(127, lambda: ln_unit(1))

            # ================= main slot loop =================
            cps = [None, None]        # live ctx PSUM tile per head
            ctxo = {}                 # (b, i) -> staging tile

            def scores_step(s):
                b, r = divmod(s, NI * NJ)
                i, j = divmod(r, NJ)
                sc = sc_tile()
                jc0 = b * S + j * P
                ic0 = b * S + i * 512
                for h in range(HPC):
                    nc.tensor.matmul(
                        sc[:, h * 512:(h + 1) * 512],
                        kT_sb[h * DH:(h + 1) * DH, jc0:jc0 + P],
                        qT_sb[h * DH:(h + 1) * DH, ic0:ic0 + 512])
                nc.scalar.activation(e_ring[:, s % ERD, :], sc,
                                     mybir.ActivationFunctionType.Exp)

            def ctx_step(c, h):
                b, r = divmod(c, NI * NJ)
                i, j = divmod(r, NJ)
                vt = b * NJ + j
                if j == 0:
                    cps[h] = ps_cps.tile([DH + 1, 512], F32, name=f"cps{h}",
                                         tag=f"cps{h}", bufs=1)
                nc.tensor.matmul(
                    cps[h],
                    v_sb[:, vt, h * (DH + 1):(h + 1) * (DH + 1)],
                    e_ring[:, c % ERD, h * 512:(h + 1) * 512],
                    start=(j == 0), stop=(j == NJ - 1))
                if j != NJ - 1:
                    return
                # ---- softmax normalize + stage for exchange ----
                # single fast PSUM->SBUF copy frees the cps bank; the
                # normalize chain then runs off the PE critical path
                if h == 0:
                    ctxo[(b, i)] = work.tile([P, 512], BF16, tag="ctxo",
                                             bufs=2, name=f"ctxo{b}_{i}")
                co = ctxo[(b, i)]
                cs = work.tile([DH, 512], F32, tag="cs", bufs=2)
                nc.vector.tensor_copy(cs, cps[h][0:DH, :])
                ssum = work.tile([1, 512], F32, tag="ssum", bufs=2)
                nc.vector.tensor_copy(ssum, cps[h][DH:DH + 1, :])
                rcp = work.tile([1, 512], F32, tag="rcp", bufs=2)
                nc.vector.reciprocal_approx_fast(rcp, ssum)
                rbc = work.tile([DH, 512], F32, tag="rbc", bufs=2)
                nc.gpsimd.partition_broadcast(rbc, rcp)
                nc.vector.tensor_mul(co[h * DH:(h + 1) * DH, :],
                                     cs, rbc)
                if h == HPC - 1:
                    tok0 = b * S + i * 512
                    q = tok0 // 1024
                    p0 = (tok0 % 1024) // P
                    for t4 in range(4):
                        nc.gpsimd.dma_start(
                            out=a2a[q][p0 + t4, :, :],
                            in_=co[:, t4 * P:(t4 + 1) * P])
                    if i % 2 == 1:  # quarter complete -> exchange
                        nc.gpsimd.collective_compute(
                            "AllToAll", mybir.AluOpType.bypass,
                            replica_groups=GRP,
                            ins=[a2a[q].opt()], outs=[ag[q].opt()])
                        for cc in range(CCH):
                            nc.sync.dma_start(out=ctxF[q][:, cc, :],
                                              in_=ag[q][cc, :, :])

            # prologue: full QKV(b0) (solid PE block, no ring hazards)
            for n in range(4):
                qk_unit(1, 0, n)
            for n in range(4):
                qk_unit(0, 0, n)
            for nt in range(4):
                v_unit(0, nt)

            for s in range(NSLOT + 3):
                if s < NSLOT:
                    scores_step(s)
                if 0 <= s - 2 < NSLOT:
                    ctx_step(s - 2, 0)
                if 0 <= s - 3 < NSLOT:
                    ctx_step(s - 3, 1)
                for fn in fillers.get(s, ()):
                    fn()

            # epilogue: LN(2) first (overlaps the last AllToAll), then
            # the last quarter's projection + its LayerNorm
            ln_unit(2)
            op_unit(3, 0)
            op_unit(3, 1)
            ln_unit(3)

    nc.compile()
    return nc


_NC_CACHE = {}


def _get_nc(S=2048, B=2, D=1024):
    key = (S, B, D)
    if key not in _NC_CACHE:
        _NC_CACHE[key] = build_bert_kernel(S, B, D)
    return _NC_CACHE[key]


def make_in_maps(query_tensor, key_tensor, value_tensor, Wq, bq, Wk, bk,
                 Wv, bv, Wo, bo, ln_w, ln_b):
    S, B, D = query_tensor.shape
    NTOK = S * B
    DL = (H // NCORES) * DH

    def bm(x):  # (S, B, D) -> batch-major (B*S, D) float32
        return np.ascontiguousarray(
            np.asarray(x, np.float32).transpose(1, 0, 2).reshape(NTOK, D))

    def bmT_tiled(x):  # (S, B, D) -> [CCH, B, 128, S] tile-contiguous
        a = bm(x).T.astype(BF16_NP)                      # (D, B*S)
        a = a.reshape(D // 128, 128, B, S).transpose(0, 2, 1, 3)
        return np.ascontiguousarray(a)

    xq = bm(query_tensor)
    xqT = bmT_tiled(query_tensor)
    xkT = bmT_tiled(key_tensor)
    xvT = bmT_tiled(value_tensor)
    woT = np.ascontiguousarray(
        np.asarray(Wo, np.float32).T.astype(BF16_NP)).reshape(D // 128, 128, D)
    f32 = lambda a: np.ascontiguousarray(np.asarray(a, np.float32))
    bf16T = lambda a: np.ascontiguousarray(
        np.asarray(a, np.float32).T.astype(BF16_NP))
    in_maps = []
    for c in range(NCORES):
        sl = slice(c * DL, (c + 1) * DL)
        rs = np.concatenate([xq[q * 1024 + c * 128:q * 1024 + (c + 1) * 128]
                             for q in range(NTOK // 1024)], axis=0)
        in_maps.append({
            "xqT": xqT, "xkT": xkT, "xvT": xvT,
            "wqT": bf16T(Wq[sl]), "wkT": bf16T(Wk[sl]),
            "wvT": bf16T(Wv[sl]), "woT": woT,
            "bq": f32(bq[sl]).reshape(DL, 1),
            "bk": f32(bk[sl]).reshape(DL, 1),
            "bv": f32(bv[sl]).reshape(1, DL),
            "bo": f32(bo).reshape(1, D),
            "lnw": f32(ln_w).reshape(1, D),
            "lnb": f32(ln_b).reshape(1, D),
            "resid": np.ascontiguousarray(rs),
        })
    return in_maps


def assemble_output(results, S, B, D):
    NTOK = S * B
    full = np.empty((NTOK, D), np.float32)
    for c, r in enumerate(results):
        o = r["out"]  # (512, D): 4 x 128-token quarter slices
        for q in range(NTOK // 1024):
            full[q * 1024 + c * 128:q * 1024 + (c + 1) * 128] = \
                o[q * 128:(q + 1) * 128]
    return np.ascontiguousarray(
        full.reshape(B, S, D).transpose(1, 0, 2))


def kernel(**inputs):
    S, B, D = inputs["query_tensor"].shape
    nc = _get_nc(S, B, D)
    in_maps = make_in_maps(**inputs)
    res = run_bass_kernel_spmd(nc, in_maps, list(range(NCORES)))
    return assemble_output(res.results, S, B, D)
